# revision 2
# baseline (speedup 1.0000x reference)
"""Trainium2 Bass kernel v2 for 2-layer GAT (nn_GAT_62182536511740).

Strategy (slot-major message passing, v2):
  * Host (pure indexing): greedy chunk balancing (each dst node's in-edges
    split evenly across 4 table chunks), degree-sorted block clustering,
    multi-block gather calls (4-block groups per chunk) to amortize SWDGE
    fixed overhead.  No edge dedup; padded slots gather a per-chunk sentinel
    row (h=0, es=-3e4) so their softmax weight is exactly 0.
  * 3 launches on 8 cores (SPMD):
      1) hext1: h1 = x@W1 plus per-node attention logits -> 272B row payload
      2) msg layer 1 (+ inline h2-table build)
      3) msg layer 2 (+ log_softmax)
  * Gathers: 4-queue dma_gather (int16 idx preloaded in ONE dma), 512B rows
    [h(128)|es(4)|ed(4)|junk] fp16.
  * Per call: vector es+ed add (per block segment), scalar Lrelu+Exp (w lands
    in msg cols 128:132), vector h*w; 3-stripe identity matmuls accumulate
    into a [128,396] PSUM bank per block (folded at finalize).
"""
import numpy as np

import concourse.bacc as bacc
import concourse.bass as bass
import concourse.mybir as mybir
import concourse.tile as tile
from concourse.vector_clock import ScopedClock

# ----------------------------------------------------------------------------
N_NODES = 100000
N_EDGES = 1600000
D_IN = 128
H = 4
D_HID = 32
D_OUT = 32
NEG_SLOPE = 0.2

NCORES = 8
NCHUNKS = 4
ROW = 256          # fp16 elements per table row (512 B)
PAY = 136          # payload elements per row [h(128)|es(4)|ed(4)]
MAXCALL = 8        # stripes per dma_gather call (<=1024 idxs, ucode ring cap)
GRP = 4            # blocks per call-group
SENT = -30000.0    # sentinel es value
F32 = mybir.dt.float32
F16 = mybir.dt.float16
I16 = mybir.dt.int16

# ----------------------------------------------------------------------------
# toolchain workaround: walrus rejects instructions with many sync waits.


def _split_waits(nc, max_waits=1):
    for bb in nc.main_func.blocks:
        insts = bb.instructions
        i = 0
        while i < len(insts):
            ins = insts[i]
            si = ins.sync_info
            if si is not None and si.on_wait and len(si.on_wait) > max_waits:
                waits = list(si.on_wait)
                keep = waits[-max_waits:]
                move = waits[: len(waits) - len(keep)]
                del si.on_wait[:]
                si.on_wait.extend(keep)
                new_nops = []
                for w in move:
                    nop = nc.engines[ins.engine].nop(nofuse=True)
                    nop_ins = nop.ins
                    emitted = nc.cur_bb.bb.instructions
                    assert emitted[-1] is nop_ins
                    emitted.pop()
                    if nop_ins.sync_info is None:
                        nop_ins.sync_info = mybir.SyncInfo(on_wait=[w], on_update=[])
                    else:
                        nop_ins.sync_info.on_wait.append(w)
                    new_nops.append(nop_ins)
                insts[i:i] = new_nops
                i += len(new_nops)
            i += 1


def _drain_and_barrier_split(self, tick_clock, wait_clock):
    nc = self.nc
    drain_inst = nc.sync.drain()
    wait_clock.add_sem_waits(
        drain_inst.ins, ScopedClock({None: tick_clock.global_clock})
    )
    si = drain_inst.ins.sync_info
    if si is not None and si.on_wait and len(si.on_wait) > 1:
        waits = list(si.on_wait)
        del si.on_wait[:]
        bb = nc.cur_bb.bb
        assert bb.instructions[-1] is drain_inst.ins
        bb.instructions.pop()
        for w in waits:
            nop = nc.sync.nop(nofuse=True)
            nsi = nop.ins.sync_info
            if nsi is None:
                nop.ins.sync_info = mybir.SyncInfo(on_wait=[w], on_update=[])
            else:
                nsi.on_wait.append(w)
        bb.instructions.append(drain_inst.ins)
    nc.all_engine_barrier()
    assert self.sems is not None
    popped = nc._tile_sem_poison_stack.pop()
    assert popped is self._sem_poison
    nc.clear_and_free_semaphores(list(self.sems.allocated().values()))
    nc.all_engine_barrier()


tile.TileContext._drain_and_barrier = _drain_and_barrier_split


# ----------------------------------------------------------------------------
# host planning (pure indexing)

def _balance_chunks(src, dst, n_nodes):
    """Greedy + refinement: assign each node (as source) to a chunk, keeping
    every dst's per-chunk in-edge counts flat.  Returns chunk_map, counts."""
    # CSR by source
    order = np.argsort(src, kind="stable")
    d_sorted = dst[order]
    starts = np.searchsorted(src[order], np.arange(n_nodes + 1))
    outdeg = np.diff(starts)
    q = np.bincount(dst, minlength=n_nodes).astype(np.float64) / NCHUNKS

    capmax = 32000
    cnt = np.zeros((n_nodes, NCHUNKS), np.int32)
    cap = np.zeros(NCHUNKS, np.int64)
    chunk_map = np.zeros(n_nodes, np.int64)
    proc = np.argsort(-outdeg, kind="stable")
    for s in proc:
        lo, hi = starts[s], starts[s + 1]
        if hi == lo:
            c = int(np.argmin(cap))
        else:
            ds = d_sorted[lo:hi]
            pen = (4.0 ** np.clip(cnt[ds] + 1 - q[ds][:, None], -8, 8)).sum(0)
            pen[cap >= capmax] = 1e18
            c = int(np.argmin(pen))
            cnt[ds, c] += 1
        chunk_map[s] = c
        cap[c] += 1

    # local-search refinement: move sources out of overfull cells
    for _ in range(3):
        moved = 0
        for s in range(n_nodes):
            lo, hi = starts[s], starts[s + 1]
            if hi == lo:
                continue
            ds = d_sorted[lo:hi]
            c0 = int(chunk_map[s])
            base = 4.0 ** np.clip(cnt[ds] - q[ds][:, None], -8, 8)
            gain = base[:, c0].sum() * 0.75
            add = base.sum(0) * 3.0
            add[c0] = 1e18
            add[cap >= capmax] = 1e18
            cb = int(np.argmin(add))
            if add[cb] < gain - 1e-12:
                cnt[ds, c0] -= 1
                cnt[ds, cb] += 1
                cap[c0] -= 1
                cap[cb] += 1
                chunk_map[s] = cb
                moved += 1
        if moved < n_nodes // 200:
            break
    return chunk_map, cnt


def build_plan(edge, n_nodes):
    src = np.asarray(edge[0], np.int64)
    dst = np.asarray(edge[1], np.int64)
    E = len(src)

    chunk_map, prof = _balance_chunks(src, dst, n_nodes)

    # block clustering: group nodes by (max count, argmax chunk, profile)
    smax = prof.max(1)
    amax = prof.argmax(1)
    order = np.lexsort((prof[:, 3], prof[:, 2], prof[:, 1], prof[:, 0],
                        amax, smax))

    nblk_tot = (n_nodes + 127) // 128
    NB = (nblk_tot + NCORES - 1) // NCORES
    NPC = NB * 128
    core_nodes = -np.ones((NCORES, NPC), np.int64)
    bi = 0
    for j in range(NB):
        for c_ in range(NCORES):
            core = c_ if (j % 2 == 0) else (NCORES - 1 - c_)
            if bi >= nblk_tot:
                continue
            blk = order[bi * 128:(bi + 1) * 128]
            core_nodes[core, j * 128:j * 128 + len(blk)] = blk
            bi += 1

    # table order: chunk-major (sentinel first in each chunk), then
    # (core, block, partition)
    tpos = -np.ones(n_nodes, np.int64)       # table row per node
    loc = -np.ones(n_nodes, np.int64)        # chunk-local index (>=1)
    chunk_starts = np.zeros(NCHUNKS, np.int64)
    t = 0
    for c in range(NCHUNKS):
        chunk_starts[c] = t
        t += 1                               # sentinel row
        lt = 1
        for core in range(NCORES):
            for j in range(NB):
                blk = core_nodes[core, j * 128:(j + 1) * 128]
                sel = blk[blk >= 0]
                sel = sel[chunk_map[sel] == c]
                tpos[sel] = t + np.arange(len(sel))
                loc[sel] = lt + np.arange(len(sel))
                t += len(sel)
                lt += len(sel)
        assert lt <= 32767
    NTAB = t
    assert NTAB == n_nodes + NCHUNKS

    node_core = -np.ones(n_nodes, np.int64)
    node_blk = -np.ones(n_nodes, np.int64)
    node_part = -np.ones(n_nodes, np.int64)
    for core in range(NCORES):
        cn = core_nodes[core]
        pos = np.nonzero(cn >= 0)[0]
        node_core[cn[pos]] = core
        node_blk[cn[pos]] = pos // 128
        node_part[cn[pos]] = pos % 128

    ecore = node_core[dst]
    eblk = node_blk[dst]
    epart = node_part[dst]
    echunk = chunk_map[src]

    # slot index within (core, blk, chunk, part)
    gkey = ((ecore * NB + eblk) * NCHUNKS + echunk) * 128 + epart
    eorder = np.lexsort((gkey,))
    gk_sorted = gkey[eorder]
    grp_start = np.r_[True, gk_sorted[1:] != gk_sorted[:-1]]
    idx_in_grp = np.arange(E) - np.maximum.accumulate(
        np.where(grp_start, np.arange(E), 0))
    eslot = np.empty(E, np.int64)
    eslot[eorder] = idx_in_grp

    cnt = np.zeros((NCORES, NB, NCHUNKS, 128), np.int32)
    np.add.at(cnt, (ecore, eblk, echunk, epart), 1)
    S = cnt.max(axis=(0, 3)).astype(np.int64)        # [NB, NCHUNKS] compiled
    TOTS = int(S.sum())

    # ---- static call schedule: 4-block groups, calls per (group, chunk)
    # stripes of chunk c within group g stream across its blocks.
    NG = (NB + GRP - 1) // GRP
    # stripe base of (j, c) within its group-chunk stream
    qbase = np.zeros((NB, NCHUNKS), np.int64)
    for g in range(NG):
        jlo, jhi = g * GRP, min((g + 1) * GRP, NB)
        for c in range(NCHUNKS):
            acc = 0
            for j in range(jlo, jhi):
                qbase[j, c] = acc
                acc += int(S[j, c])

    # calls: list of dicts (g, c, ns, col, segs=[(s0, s1, j, q0)])
    calls = []
    col = 0
    for g in range(NG):
        jlo, jhi = g * GRP, min((g + 1) * GRP, NB)
        for c in range(NCHUNKS):
            ts = sum(int(S[j, c]) for j in range(jlo, jhi))
            k = 0
            while k < ts:
                ns = min(MAXCALL, ts - k)
                # segments of this call
                segs = []
                for j in range(jlo, jhi):
                    b0, b1 = int(qbase[j, c]), int(qbase[j, c] + S[j, c])
                    s0 = max(b0, k) - k
                    s1 = min(b1, k + ns) - k
                    if s1 > s0:
                        segs.append((s0, s1, j, max(b0, k) - b0))
                calls.append(dict(g=g, c=c, ns=ns, col=col, segs=segs))
                col += ns * 8
                k += ns
    IWTOT = col
    NCALLS = len(calls)

    # per-block last-matmul bookkeeping: last (call index, seg index) per block
    last_of_block = {}
    for ci, cl in enumerate(calls):
        for si_, (s0, s1, j, q0) in enumerate(cl["segs"]):
            last_of_block[j] = (ci, si_)

    # ---- per-core idx tables [128, IWTOT] int16 (16-wrap, x8 replicated)
    # call-position value: slot (stripe k within call, part p) at col k*128+p
    idx_tab = np.zeros((NCORES, 128, IWTOT), np.int16)
    # map each edge to (core, call col position)
    # stripe within group-chunk stream = qbase[j,c] + eslot
    estripe = qbase[eblk, echunk] + eslot
    # call index within (g, c): precompute per (g,c) col bases & stripe starts
    callmeta = {}
    for ci, cl in enumerate(calls):
        callmeta.setdefault((cl["g"], cl["c"]), []).append(ci)
    # for vector lookup: per (g, c), stripes split in groups of 8
    egrp = eblk // GRP
    ecall_k = estripe // MAXCALL      # which call within (g,c) stream
    ecall_s = estripe % MAXCALL       # stripe within call
    # col base per (g, c, k)
    colbase = {}
    for (g, c), cis in callmeta.items():
        for k, ci in enumerate(cis):
            colbase[(g, c, k)] = calls[ci]["col"]
    ecol = np.fromiter(
        (colbase[(int(g_), int(c_), int(k_))] for g_, c_, k_ in
         zip(egrp, echunk, ecall_k)),
        np.int64, count=E)
    # position within call = stripe*128 + part ; value = chunk-local row
    epos = ecall_s * 128 + epart
    # flat per-core fill
    for core in range(NCORES):
        esel = np.nonzero(ecore == core)[0]
        flat = np.zeros((IWTOT // 8) * 128, np.int64)  # positions per call run
        # column in 16-wrap layout: call col + (pos // 16) ... build via
        # full flat position: fpos = ecol*16 + epos  (each col covers 16 pos)
        fpos = ecol[esel] * 16 + epos[esel]
        vals = loc[src[esel]]
        flat_full = np.zeros(IWTOT * 16, np.int64)
        flat_full[fpos] = vals
        wrap = flat_full.reshape(IWTOT, 16).T.astype(np.int16)  # [16, IWTOT]
        idx_tab[core] = np.tile(wrap, (8, 1))

    return dict(
        tpos=tpos, loc=loc, core_nodes=core_nodes, chunk_starts=chunk_starts,
        NB=NB, NPC=NPC, NG=NG, S=S, calls=calls, IWTOT=IWTOT, TOTS=TOTS,
        NCALLS=NCALLS, idx_tab=idx_tab, last_of_block=last_of_block,
        n_nodes=n_nodes, NTAB=NTAB,
        block_ts=S.sum(1),
    )


# ----------------------------------------------------------------------------
# bass builders

def build_hext(seg_len):
    """Launch 1: per core computes table payload rows for seg_len nodes.

    inputs : xT [128, seg_len] fp16, Wt [128,128] fp16,
             as_rep [128,128] fp32, ad_rep [128,128] fp32
    output : hx [seg_len, PAY] fp16  rows = [h(128) | es(4) | ed(4)]
    """
    nc = bacc.Bacc("TRN2", num_swdge_queues=4)
    xT = nc.dram_tensor("xT", [128, seg_len], F16, kind="ExternalInput")
    Wt = nc.dram_tensor("Wt", [128, 128], F16, kind="ExternalInput")
    as_rep = nc.dram_tensor("as_rep", [128, 128], F32, kind="ExternalInput")
    ad_rep = nc.dram_tensor("ad_rep", [128, 128], F32, kind="ExternalInput")
    hx = nc.dram_tensor("hx", [seg_len, PAY], F16, kind="ExternalOutput")

    ntiles = (seg_len + 127) // 128
    with tile.TileContext(nc) as tc:
        with (
            tc.tile_pool(name="consts", bufs=1) as cpool,
            tc.tile_pool(name="work", bufs=4) as pool,
            tc.tile_pool(name="ps", bufs=4, space="PSUM") as pp,
        ):
            wt = cpool.tile([128, 128], F16)
            nc.sync.dma_start(out=wt[:], in_=Wt[:])
            asr = cpool.tile([128, 128], F32)
            nc.sync.dma_start(out=asr[:], in_=as_rep[:])
            adr = cpool.tile([128, 128], F32)
            nc.sync.dma_start(out=adr[:], in_=ad_rep[:])
            for t in range(ntiles):
                nt = min(128, seg_len - t * 128)
                xt = pool.tile([128, 128], F16, tag="xt")
                nc.sync.dma_start(out=xt[:, :nt], in_=xT[:, t * 128:t * 128 + nt])
                ph = pp.tile([128, 128], F32)
                nc.tensor.matmul(ph[:nt, :], lhsT=xt[:, :nt], rhs=wt[:],
                                 start=True, stop=True)
                row = pool.tile([128, PAY], F16, tag="row")
                nc.vector.tensor_copy(row[:nt, 0:128], ph[:nt, :])
                scr = pool.tile([128, 32], F32, tag="scr")
                for h in range(H):
                    nc.vector.scalar_tensor_tensor(
                        out=scr[:nt, :], in0=ph[:nt, h * 32:(h + 1) * 32],
                        scalar=1.0, in1=asr[:nt, h * 32:(h + 1) * 32],
                        op0=mybir.AluOpType.mult, op1=mybir.AluOpType.mult,
                        accum_out=row[:nt, 128 + h:129 + h])
                for h in range(H):
                    nc.vector.scalar_tensor_tensor(
                        out=scr[:nt, :], in0=ph[:nt, h * 32:(h + 1) * 32],
                        scalar=1.0, in1=adr[:nt, h * 32:(h + 1) * 32],
                        op0=mybir.AluOpType.mult, op1=mybir.AluOpType.mult,
                        accum_out=row[:nt, 132 + h:133 + h])
                nc.sync.dma_start(out=hx[t * 128:t * 128 + nt, :], in_=row[:nt, :])
    nc.compile()
    _split_waits(nc, max_waits=1)
    return nc


def build_msg(plan, layer2):
    """Launch 2/3: slot-major message passing for one layer on each core.

    inputs : tab [NTAB, ROW] fp16, idxs [128, IWTOT] int16,
             edt_all [128, NB*4] fp16, btile [128,128] fp32,
             ident [128,128] fp16,
             (layer1) W2t [128,128] fp16, a2s_rep/a2d_rep [128,128] fp32
    output : layer1: hx2 [NPC, PAY] fp16 ; layer2: outp [NPC, 128] fp32
    """
    NB, NG, S, calls = plan["NB"], plan["NG"], plan["S"], plan["calls"]
    NPC, IWTOT = plan["NPC"], plan["IWTOT"]
    NTAB = plan["NTAB"]
    cs = plan["chunk_starts"]
    last_of_block = plan["last_of_block"]
    block_ts = plan["block_ts"]

    nc = bacc.Bacc("TRN2", num_swdge_queues=4)
    tab = nc.dram_tensor("tab", [NTAB, ROW], F16, kind="ExternalInput")
    idxs = nc.dram_tensor("idxs", [128, IWTOT], I16, kind="ExternalInput")
    eds = nc.dram_tensor("eds", [128, NB * 4], F16, kind="ExternalInput")
    btile = nc.dram_tensor("btile", [128, 128], F32, kind="ExternalInput")
    identt = nc.dram_tensor("ident", [128, 128], F16, kind="ExternalInput")
    if not layer2:
        W2t = nc.dram_tensor("W2t", [128, 128], F16, kind="ExternalInput")
        a2s = nc.dram_tensor("a2s_rep", [128, 128], F32, kind="ExternalInput")
        a2d = nc.dram_tensor("a2d_rep", [128, 128], F32, kind="ExternalInput")
        hx2 = nc.dram_tensor("hx2", [NPC, PAY], F16, kind="ExternalOutput")
    else:
        outp = nc.dram_tensor("outp", [NPC, 128], F32, kind="ExternalOutput")

    # chunk sizes for gather source windows
    csz = [int((cs[c + 1] if c + 1 < NCHUNKS else NTAB) - cs[c])
           for c in range(NCHUNKS)]

    A = mybir.AluOpType
    AF = mybir.ActivationFunctionType
    qn = 0
    with tile.TileContext(nc) as tc:
        with (
            tc.tile_pool(name="consts", bufs=1) as cpool,
            tc.tile_pool(name="gath", bufs=8) as gp,
            tc.tile_pool(name="wp", bufs=8) as wp,
            tc.tile_pool(name="msgp", bufs=8) as mp,
            tc.tile_pool(name="finp", bufs=3) as fp_,
            tc.tile_pool(name="psb", bufs=6, space="PSUM") as ppb,
            tc.tile_pool(name="psx", bufs=2, space="PSUM") as ppx,
        ):
            ident = cpool.tile([128, 128], F16)
            nc.sync.dma_start(out=ident[:], in_=identt[:])
            bt = cpool.tile([128, 128], F32)
            nc.sync.dma_start(out=bt[:], in_=btile[:])
            edt = cpool.tile([128, NB * 4], F16)
            nc.sync.dma_start(out=edt[:], in_=eds[:])
            itab = cpool.tile([128, IWTOT], I16)
            nc.sync.dma_start(out=itab[:], in_=idxs[:])
            zt = cpool.tile([128, 3 * 132], F16)
            nc.vector.memset(zt[:], 0.0)
            if not layer2:
                w2 = cpool.tile([128, 128], F16)
                nc.sync.dma_start(out=w2[:], in_=W2t[:])
                a2sr = cpool.tile([128, 128], F32)
                nc.sync.dma_start(out=a2sr[:], in_=a2s[:])
                a2dr = cpool.tile([128, 128], F32)
                nc.sync.dma_start(out=a2dr[:], in_=a2d[:])
                # device-side pack: w2e = [W2 | W2@bd(a2s) | W2@bd(a2d)]
                w2e = cpool.tile([128, PAY], F16)
                nc.vector.tensor_copy(w2e[:, 0:128], w2[:])
                pscr = cpool.tile([128, 32], F32)
                for h in range(H):
                    nc.vector.scalar_tensor_tensor(
                        out=pscr[:], in0=w2[:, h * 32:(h + 1) * 32], scalar=1.0,
                        in1=a2sr[:, h * 32:(h + 1) * 32],
                        op0=A.mult, op1=A.mult,
                        accum_out=w2e[:, 128 + h:129 + h])
                for h in range(H):
                    nc.vector.scalar_tensor_tensor(
                        out=pscr[:], in0=w2[:, h * 32:(h + 1) * 32], scalar=1.0,
                        in1=a2dr[:, h * 32:(h + 1) * 32],
                        op0=A.mult, op1=A.mult,
                        accum_out=w2e[:, 132 + h:133 + h])

            pb_of = {}       # open PSUM tile per block
            ci = 0
            for g in range(NG):
                jlo, jhi = g * GRP, min((g + 1) * GRP, NB)
                for j in range(jlo, jhi):
                    if block_ts[j] == 0:
                        continue
                    pb = ppb.tile([128, 3 * 132], F32, tag="pb")
                    pb_of[j] = pb
                    nc.tensor.matmul(pb[:], lhsT=ident[:], rhs=zt[:],
                                     start=True, stop=False)
                while ci < len(calls) and calls[ci]["g"] == g:
                    cl = calls[ci]
                    c, ns, col, segs = cl["c"], cl["ns"], cl["col"], cl["segs"]
                    gt = gp.tile([128, MAXCALL * ROW], F16, tag="gt")
                    nc.gpsimd.dma_gather(
                        gt[:, :ns * ROW].rearrange("p (k e) -> p k e", e=ROW),
                        tab[int(cs[c]):int(cs[c]) + csz[c], :],
                        itab[:, col:col + ns * 8], ns * 128, ns * 128, ROW,
                        single_packet=False, queue_num=qn % 4)
                    qn += 1
                    gv = gt[:, :ns * ROW].rearrange("p (k e) -> p k e", e=ROW)
                    # logits: lg = es + ed (per block segment)
                    wt_ = wp.tile([128, MAXCALL * 4], F32, tag="wt")
                    for (s0, s1, j, q0) in segs:
                        _e = edt[:, j * 4:(j + 1) * 4]
                        nc.vector.tensor_tensor(
                            out=wt_[:, s0 * 4:s1 * 4].rearrange(
                                "p (k e) -> p k e", e=4),
                            in0=gv[:, s0:s1, 128:132],
                            in1=bass.AP(_e.tensor, _e.offset,
                                        [_e.ap[0], [0, s1 - s0], [1, 4]]),
                            op=A.add)
                    # w = exp(lrelu(lg)) ; w lands in msg cols 128:132
                    nc.vector.scalar_tensor_tensor(
                        out=wt_[:, :ns * 4], in0=wt_[:, :ns * 4],
                        scalar=NEG_SLOPE, in1=wt_[:, :ns * 4],
                        op0=A.mult, op1=A.max)
                    msg = mp.tile([128, MAXCALL * 132], F16, tag="msg")
                    msg_v = msg[:, :ns * 132].rearrange("p (k e) -> p k e", e=132)
                    nc.scalar.activation(
                        msg_v[:, :, 128:132],
                        wt_[:, :ns * 4].rearrange("p (k e) -> p k e", e=4),
                        AF.Exp)
                    # msg = h * w
                    wv = msg_v[:, :, 128:132]
                    nc.vector.tensor_tensor(
                        out=msg_v[:, :, 0:128].rearrange(
                            "p k (h d) -> p k h d", d=32),
                        in0=gv[:, :, 0:128].rearrange("p k (h d) -> p k h d", d=32),
                        in1=bass.AP(wv.tensor, wv.offset,
                                    [wv.ap[0], [132, ns], [1, 4], [0, 32]]),
                        op=A.mult)
                    # 3-stripe accumulating matmuls per segment
                    for si_, (s0, s1, j, q0) in enumerate(segs):
                        pb = pb_of[j]
                        is_last_seg = last_of_block[j] == (ci, si_)
                        t0 = s0
                        while t0 < s1:
                            te = min(t0 + 3, s1)
                            stop = is_last_seg and te == s1
                            nc.tensor.matmul(
                                pb[:, :(te - t0) * 132], lhsT=ident[:],
                                rhs=msg[:, t0 * 132:te * 132],
                                start=False, stop=stop)
                            t0 = te
                    ci += 1
                # finalize blocks of this group
                for j in range(jlo, jhi):
                    if block_ts[j] == 0:
                        continue
                    pb = pb_of.pop(j)
                    acc = fp_.tile([128, 132], F32, tag="acc")
                    nc.vector.tensor_copy(acc[:], pb[:, 0:132])
                    nc.vector.tensor_tensor(out=acc[:], in0=acc[:],
                                            in1=pb[:, 132:264], op=A.add)
                    nc.vector.tensor_tensor(out=acc[:], in0=acc[:],
                                            in1=pb[:, 264:396], op=A.add)
                    den = fp_.tile([128, 4], F32, tag="den")
                    nc.vector.tensor_scalar_add(den[:], acc[:, 128:132], 1e-20)
                    nc.vector.reciprocal(den[:], den[:])
                    t1 = fp_.tile([128, 128], F32, tag="t1")
                    nc.vector.tensor_tensor(
                        out=t1[:].rearrange("p (h d) -> p h d", d=32),
                        in0=acc[:, 0:128].rearrange("p (h d) -> p h d", d=32),
                        in1=bass.AP(den.tensor, den.offset,
                                    [den.ap[0], [1, 4], [0, 32]]),
                        op=A.mult)
                    nc.vector.tensor_tensor(out=t1[:], in0=t1[:], in1=bt[:],
                                            op=A.add)
                    if not layer2:
                        x2 = fp_.tile([128, 128], F16, tag="x2")
                        nc.vector.tensor_scalar_max(x2[:], t1[:], 0.0)
                        px = ppx.tile([128, 128], F16, tag="tx")
                        nc.tensor.transpose(px[:], x2[:], ident[:])
                        x2t = fp_.tile([128, 128], F16, tag="x2t")
                        nc.vector.tensor_copy(x2t[:], px[:])
                        ph2 = ppx.tile([128, PAY], F32, tag="tx")
                        nc.tensor.matmul(ph2[:], lhsT=x2t[:], rhs=w2e[:],
                                         start=True, stop=True)
                        row = fp_.tile([128, PAY], F16, tag="row")
                        nc.vector.tensor_copy(row[:], ph2[:])
                        nc.sync.dma_start(out=hx2[j * 128:(j + 1) * 128, :],
                                          in_=row[:])
                    else:
                        et = fp_.tile([128, 128], F32, tag="et")
                        nc.scalar.activation(et[:], t1[:], AF.Exp)
                        ssum = fp_.tile([128, 1], F32, tag="ssum")
                        nc.vector.tensor_reduce(ssum[:], et[:],
                                                axis=mybir.AxisListType.X,
                                                op=A.add)
                        nc.scalar.activation(ssum[:], ssum[:], AF.Ln)
                        nc.vector.tensor_scalar_mul(ssum[:], ssum[:], -1.0)
                        to = fp_.tile([128, 128], F32, tag="to")
                        nc.scalar.activation(to[:], t1[:], AF.Identity,
                                             bias=ssum[:, 0:1])
                        nc.sync.dma_start(out=outp[j * 128:(j + 1) * 128, :],
                                          in_=to[:])
    nc.compile()
    _split_waits(nc, max_waits=1)
    return nc


# ----------------------------------------------------------------------------
# runner

def _rep_heads(a):
    return np.tile(np.asarray(a).reshape(1, -1).astype(np.float32), (128, 1))


def _run(nc, in_maps):
    from concourse.bass_utils import run_bass_kernel_spmd
    return run_bass_kernel_spmd(nc, in_maps, core_ids=list(range(NCORES)),
                                trace=False).results


def _assemble_tab(plan, hx_by_core, core_seg_nodes):
    """hx rows (per-core, perm order) -> full table with sentinels."""
    n = plan["n_nodes"]
    tab = np.zeros((plan["NTAB"], ROW), np.float16)
    tab[plan["chunk_starts"], 128:132] = SENT
    tpos = plan["tpos"]
    for core in range(NCORES):
        nodes = core_seg_nodes[core]
        tab[tpos[nodes], :PAY] = hx_by_core[core]
    return tab


def _tab_from_blocks(plan, hx2_by_core):
    """hx2 rows (per-core, block order) -> full table with sentinels."""
    tab = np.zeros((plan["NTAB"], ROW), np.float16)
    tab[plan["chunk_starts"], 128:132] = SENT
    tpos = plan["tpos"]
    core_nodes = plan["core_nodes"]
    for core in range(NCORES):
        cn = core_nodes[core]
        vm = cn >= 0
        tab[tpos[cn[vm]], :PAY] = np.asarray(hx2_by_core[core])[vm]
    return tab


def _eds_of(plan, tab):
    """per-core [128, NB*4] fp16 ed table in (part, block*4) layout."""
    NB = plan["NB"]
    core_nodes = plan["core_nodes"]
    tpos = plan["tpos"]
    eds = np.zeros((NCORES, 128, NB * 4), np.float16)
    for core in range(NCORES):
        cn = core_nodes[core].reshape(NB, 128)     # [j, p]
        vm = cn >= 0
        vals = np.zeros((NB, 128, 4), np.float16)
        vals[vm] = tab[tpos[cn[vm]], 132:136]
        eds[core] = vals.transpose(1, 0, 2).reshape(128, NB * 4)
    return eds


def run_pipeline(inputs, n_nodes, run=_run):
    edge = np.asarray(inputs["edge"])
    x = np.asarray(inputs["features"], np.float32)
    W1 = np.asarray(inputs["W1"], np.float32)
    a1s = np.asarray(inputs["a1_src"], np.float32)
    a1d = np.asarray(inputs["a1_dst"], np.float32)
    b1 = np.asarray(inputs["b1"], np.float32)
    W2 = np.asarray(inputs["W2"], np.float32)
    a2s = np.asarray(inputs["a2_src"], np.float32)
    a2d = np.asarray(inputs["a2_dst"], np.float32)
    b2 = np.asarray(inputs["b2"], np.float32)

    plan = build_plan(edge, n_nodes)
    NB, NPC = plan["NB"], plan["NPC"]
    core_nodes = plan["core_nodes"]

    # ---- launch 1: hext1 over nodes in table order split across cores
    tord = np.argsort(plan["tpos"])          # nodes in table-row order
    seg = (n_nodes + NCORES - 1) // NCORES
    pad = seg * NCORES - n_nodes
    tord_p = np.concatenate([tord, tord[:pad]]) if pad else tord
    nc1 = build_hext(seg)
    in1, seg_nodes_of = [], []
    for core in range(NCORES):
        seg_nodes = tord_p[core * seg:(core + 1) * seg]
        seg_nodes_of.append(seg_nodes)
        xT = np.ascontiguousarray(x[seg_nodes].astype(np.float16).T)
        in1.append({
            "xT": xT, "Wt": W1.astype(np.float16),
            "as_rep": _rep_heads(a1s), "ad_rep": _rep_heads(a1d),
        })
    res1 = run(nc1, in1)
    hx_by_core = [np.asarray(res1[c]["hx"]) for c in range(NCORES)]
    # dedupe the pad overlap: later writes win, identical rows anyway
    tab1 = _assemble_tab(plan, hx_by_core, seg_nodes_of)
    eds1 = _eds_of(plan, tab1)

    ident = np.eye(128, dtype=np.float16)

    # ---- launch 2: layer-1 message passing + inline h2 table rows
    nc2 = build_msg(plan, layer2=False)
    in2 = []
    for core in range(NCORES):
        in2.append({
            "tab": tab1, "idxs": plan["idx_tab"][core],
            "eds": eds1[core],
            "btile": np.tile(b1.reshape(1, -1), (128, 1)).astype(np.float32),
            "ident": ident, "W2t": W2.astype(np.float16),
            "a2s_rep": _rep_heads(a2s), "a2d_rep": _rep_heads(a2d),
        })
    res2 = run(nc2, in2)
    tab2 = _tab_from_blocks(plan, [res2[c]["hx2"] for c in range(NCORES)])
    eds2 = _eds_of(plan, tab2)

    # ---- launch 3: layer-2 message passing + log_softmax
    nc3 = build_msg(plan, layer2=True)
    in3 = []
    for core in range(NCORES):
        in3.append({
            "tab": tab2, "idxs": plan["idx_tab"][core],
            "eds": eds2[core],
            "btile": np.tile(b2.reshape(1, -1), (128, 1)).astype(np.float32),
            "ident": ident,
        })
    res3 = run(nc3, in3)

    out = np.zeros((n_nodes, H * D_OUT), np.float32)
    for core in range(NCORES):
        cn = core_nodes[core]
        vm = cn >= 0
        out[cn[vm]] = np.asarray(res3[core]["outp"])[vm]
    return out


def kernel(**inputs):
    return run_pipeline(inputs, N_NODES).astype(np.float32)


# revision 3
# speedup vs baseline: 1.1204x; 1.1204x over previous
"""Trainium2 Bass kernel v2 for 2-layer GAT (nn_GAT_62182536511740).

Strategy (slot-major message passing, v2):
  * Host (pure indexing): greedy chunk balancing (each dst node's in-edges
    split evenly across 4 table chunks), degree-sorted block clustering,
    multi-block gather calls (4-block groups per chunk) to amortize SWDGE
    fixed overhead.  No edge dedup; padded slots gather a per-chunk sentinel
    row (h=0, es=-3e4) so their softmax weight is exactly 0.
  * 3 launches on 8 cores (SPMD):
      1) hext1: h1 = x@W1 plus per-node attention logits -> 272B row payload
      2) msg layer 1 (+ inline h2-table build)
      3) msg layer 2 (+ log_softmax)
  * Gathers: 4-queue dma_gather (int16 idx preloaded in ONE dma), 512B rows
    [h(128)|es(4)|ed(4)|junk] fp16.
  * Per call: vector es+ed add (per block segment), scalar Lrelu+Exp (w lands
    in msg cols 128:132), vector h*w; 3-stripe identity matmuls accumulate
    into a [128,396] PSUM bank per block (folded at finalize).
"""
import numpy as np

import concourse.bacc as bacc
import concourse.bass as bass
import concourse.mybir as mybir
import concourse.tile as tile
from concourse.vector_clock import ScopedClock

# ----------------------------------------------------------------------------
N_NODES = 100000
N_EDGES = 1600000
D_IN = 128
H = 4
D_HID = 32
D_OUT = 32
NEG_SLOPE = 0.2

NCORES = 8
NCHUNKS = 4
ROW = 256          # fp16 elements per table row (512 B)
PAY = 136          # payload elements per row [h(128)|es(4)|ed(4)]
MAXCALL = 8        # stripes per dma_gather call (<=1024 idxs, ucode ring cap)
GRP = 4            # blocks per call-group
SENT = -30000.0    # sentinel es value
F32 = mybir.dt.float32
F16 = mybir.dt.float16
I16 = mybir.dt.int16

# ----------------------------------------------------------------------------
# toolchain workaround: walrus rejects instructions with many sync waits.


def _split_waits(nc, max_waits=1):
    for bb in nc.main_func.blocks:
        insts = bb.instructions
        i = 0
        while i < len(insts):
            ins = insts[i]
            si = ins.sync_info
            if si is not None and si.on_wait and len(si.on_wait) > max_waits:
                waits = list(si.on_wait)
                keep = waits[-max_waits:]
                move = waits[: len(waits) - len(keep)]
                del si.on_wait[:]
                si.on_wait.extend(keep)
                new_nops = []
                for w in move:
                    nop = nc.engines[ins.engine].nop(nofuse=True)
                    nop_ins = nop.ins
                    emitted = nc.cur_bb.bb.instructions
                    assert emitted[-1] is nop_ins
                    emitted.pop()
                    if nop_ins.sync_info is None:
                        nop_ins.sync_info = mybir.SyncInfo(on_wait=[w], on_update=[])
                    else:
                        nop_ins.sync_info.on_wait.append(w)
                    new_nops.append(nop_ins)
                insts[i:i] = new_nops
                i += len(new_nops)
            i += 1


def _drain_and_barrier_split(self, tick_clock, wait_clock):
    nc = self.nc
    drain_inst = nc.sync.drain()
    wait_clock.add_sem_waits(
        drain_inst.ins, ScopedClock({None: tick_clock.global_clock})
    )
    si = drain_inst.ins.sync_info
    if si is not None and si.on_wait and len(si.on_wait) > 1:
        waits = list(si.on_wait)
        del si.on_wait[:]
        bb = nc.cur_bb.bb
        assert bb.instructions[-1] is drain_inst.ins
        bb.instructions.pop()
        for w in waits:
            nop = nc.sync.nop(nofuse=True)
            nsi = nop.ins.sync_info
            if nsi is None:
                nop.ins.sync_info = mybir.SyncInfo(on_wait=[w], on_update=[])
            else:
                nsi.on_wait.append(w)
        bb.instructions.append(drain_inst.ins)
    nc.all_engine_barrier()
    assert self.sems is not None
    popped = nc._tile_sem_poison_stack.pop()
    assert popped is self._sem_poison
    nc.clear_and_free_semaphores(list(self.sems.allocated().values()))
    nc.all_engine_barrier()


tile.TileContext._drain_and_barrier = _drain_and_barrier_split


# ----------------------------------------------------------------------------
# host planning (pure indexing)

def _balance_chunks(src, dst, n_nodes):
    """Greedy + refinement: assign each node (as source) to a chunk, keeping
    every dst's per-chunk in-edge counts flat.  Returns chunk_map, counts."""
    # CSR by source
    order = np.argsort(src, kind="stable")
    d_sorted = dst[order]
    starts = np.searchsorted(src[order], np.arange(n_nodes + 1))
    outdeg = np.diff(starts)
    q = np.bincount(dst, minlength=n_nodes).astype(np.float64) / NCHUNKS

    capmax = 32000
    cnt = np.zeros((n_nodes, NCHUNKS), np.int32)
    cap = np.zeros(NCHUNKS, np.int64)
    chunk_map = np.zeros(n_nodes, np.int64)
    proc = np.argsort(-outdeg, kind="stable")
    for s in proc:
        lo, hi = starts[s], starts[s + 1]
        if hi == lo:
            c = int(np.argmin(cap))
        else:
            ds = d_sorted[lo:hi]
            pen = (4.0 ** np.clip(cnt[ds] + 1 - q[ds][:, None], -8, 8)).sum(0)
            pen[cap >= capmax] = 1e18
            c = int(np.argmin(pen))
            cnt[ds, c] += 1
        chunk_map[s] = c
        cap[c] += 1

    # local-search refinement: move sources out of overfull cells
    for _ in range(3):
        moved = 0
        for s in range(n_nodes):
            lo, hi = starts[s], starts[s + 1]
            if hi == lo:
                continue
            ds = d_sorted[lo:hi]
            c0 = int(chunk_map[s])
            base = 4.0 ** np.clip(cnt[ds] - q[ds][:, None], -8, 8)
            gain = base[:, c0].sum() * 0.75
            add = base.sum(0) * 3.0
            add[c0] = 1e18
            add[cap >= capmax] = 1e18
            cb = int(np.argmin(add))
            if add[cb] < gain - 1e-12:
                cnt[ds, c0] -= 1
                cnt[ds, cb] += 1
                cap[c0] -= 1
                cap[cb] += 1
                chunk_map[s] = cb
                moved += 1
        if moved < n_nodes // 200:
            break
    return chunk_map, cnt


def build_plan(edge, n_nodes):
    src = np.asarray(edge[0], np.int64)
    dst = np.asarray(edge[1], np.int64)
    E = len(src)

    chunk_map, prof = _balance_chunks(src, dst, n_nodes)

    # block clustering: group nodes by (max count, argmax chunk, profile)
    smax = prof.max(1)
    amax = prof.argmax(1)
    order = np.lexsort((prof[:, 3], prof[:, 2], prof[:, 1], prof[:, 0],
                        amax, smax))

    nblk_tot = (n_nodes + 127) // 128
    NB = (nblk_tot + NCORES - 1) // NCORES
    NPC = NB * 128
    core_nodes = -np.ones((NCORES, NPC), np.int64)
    bi = 0
    for j in range(NB):
        for c_ in range(NCORES):
            core = c_ if (j % 2 == 0) else (NCORES - 1 - c_)
            if bi >= nblk_tot:
                continue
            blk = order[bi * 128:(bi + 1) * 128]
            core_nodes[core, j * 128:j * 128 + len(blk)] = blk
            bi += 1

    # table order: chunk-major (sentinel first in each chunk), then
    # (core, block, partition)
    tpos = -np.ones(n_nodes, np.int64)       # table row per node
    loc = -np.ones(n_nodes, np.int64)        # chunk-local index (>=1)
    chunk_starts = np.zeros(NCHUNKS, np.int64)
    t = 0
    for c in range(NCHUNKS):
        chunk_starts[c] = t
        t += 1                               # sentinel row
        lt = 1
        for core in range(NCORES):
            for j in range(NB):
                blk = core_nodes[core, j * 128:(j + 1) * 128]
                sel = blk[blk >= 0]
                sel = sel[chunk_map[sel] == c]
                tpos[sel] = t + np.arange(len(sel))
                loc[sel] = lt + np.arange(len(sel))
                t += len(sel)
                lt += len(sel)
        assert lt <= 32767
    NTAB = t
    assert NTAB == n_nodes + NCHUNKS

    node_core = -np.ones(n_nodes, np.int64)
    node_blk = -np.ones(n_nodes, np.int64)
    node_part = -np.ones(n_nodes, np.int64)
    for core in range(NCORES):
        cn = core_nodes[core]
        pos = np.nonzero(cn >= 0)[0]
        node_core[cn[pos]] = core
        node_blk[cn[pos]] = pos // 128
        node_part[cn[pos]] = pos % 128

    ecore = node_core[dst]
    eblk = node_blk[dst]
    epart = node_part[dst]
    echunk = chunk_map[src]

    # slot index within (core, blk, chunk, part)
    gkey = ((ecore * NB + eblk) * NCHUNKS + echunk) * 128 + epart
    eorder = np.lexsort((gkey,))
    gk_sorted = gkey[eorder]
    grp_start = np.r_[True, gk_sorted[1:] != gk_sorted[:-1]]
    idx_in_grp = np.arange(E) - np.maximum.accumulate(
        np.where(grp_start, np.arange(E), 0))
    eslot = np.empty(E, np.int64)
    eslot[eorder] = idx_in_grp

    cnt = np.zeros((NCORES, NB, NCHUNKS, 128), np.int32)
    np.add.at(cnt, (ecore, eblk, echunk, epart), 1)
    S = cnt.max(axis=(0, 3)).astype(np.int64)        # [NB, NCHUNKS] compiled
    TOTS = int(S.sum())

    # ---- static call schedule: 4-block groups, calls per (group, chunk)
    # stripes of chunk c within group g stream across its blocks.
    NG = (NB + GRP - 1) // GRP
    # stripe base of (j, c) within its group-chunk stream
    qbase = np.zeros((NB, NCHUNKS), np.int64)
    for g in range(NG):
        jlo, jhi = g * GRP, min((g + 1) * GRP, NB)
        for c in range(NCHUNKS):
            acc = 0
            for j in range(jlo, jhi):
                qbase[j, c] = acc
                acc += int(S[j, c])

    # calls: list of dicts (g, c, ns, col, segs=[(s0, s1, j, q0)])
    calls = []
    col = 0
    for g in range(NG):
        jlo, jhi = g * GRP, min((g + 1) * GRP, NB)
        for c in range(NCHUNKS):
            ts = sum(int(S[j, c]) for j in range(jlo, jhi))
            k = 0
            while k < ts:
                ns = min(MAXCALL, ts - k)
                # segments of this call
                segs = []
                for j in range(jlo, jhi):
                    b0, b1 = int(qbase[j, c]), int(qbase[j, c] + S[j, c])
                    s0 = max(b0, k) - k
                    s1 = min(b1, k + ns) - k
                    if s1 > s0:
                        segs.append((s0, s1, j, max(b0, k) - b0))
                calls.append(dict(g=g, c=c, ns=ns, col=col, segs=segs))
                col += ns * 8
                k += ns
    IWTOT = col
    NCALLS = len(calls)

    # per-block last-matmul bookkeeping: last (call index, seg index) per block
    last_of_block = {}
    for ci, cl in enumerate(calls):
        for si_, (s0, s1, j, q0) in enumerate(cl["segs"]):
            last_of_block[j] = (ci, si_)

    # ---- per-core idx tables [128, IWTOT] int16 (16-wrap, x8 replicated)
    # call-position value: slot (stripe k within call, part p) at col k*128+p
    idx_tab = np.zeros((NCORES, 128, IWTOT), np.int16)
    # map each edge to (core, call col position)
    # stripe within group-chunk stream = qbase[j,c] + eslot
    estripe = qbase[eblk, echunk] + eslot
    # call index within (g, c): precompute per (g,c) col bases & stripe starts
    callmeta = {}
    for ci, cl in enumerate(calls):
        callmeta.setdefault((cl["g"], cl["c"]), []).append(ci)
    # for vector lookup: per (g, c), stripes split in groups of 8
    egrp = eblk // GRP
    ecall_k = estripe // MAXCALL      # which call within (g,c) stream
    ecall_s = estripe % MAXCALL       # stripe within call
    # col base per (g, c, k)
    colbase = {}
    for (g, c), cis in callmeta.items():
        for k, ci in enumerate(cis):
            colbase[(g, c, k)] = calls[ci]["col"]
    ecol = np.fromiter(
        (colbase[(int(g_), int(c_), int(k_))] for g_, c_, k_ in
         zip(egrp, echunk, ecall_k)),
        np.int64, count=E)
    # position within call = stripe*128 + part ; value = chunk-local row
    epos = ecall_s * 128 + epart
    # flat per-core fill
    for core in range(NCORES):
        esel = np.nonzero(ecore == core)[0]
        flat = np.zeros((IWTOT // 8) * 128, np.int64)  # positions per call run
        # column in 16-wrap layout: call col + (pos // 16) ... build via
        # full flat position: fpos = ecol*16 + epos  (each col covers 16 pos)
        fpos = ecol[esel] * 16 + epos[esel]
        vals = loc[src[esel]]
        flat_full = np.zeros(IWTOT * 16, np.int64)
        flat_full[fpos] = vals
        wrap = flat_full.reshape(IWTOT, 16).T.astype(np.int16)  # [16, IWTOT]
        idx_tab[core] = np.tile(wrap, (8, 1))

    return dict(
        tpos=tpos, loc=loc, core_nodes=core_nodes, chunk_starts=chunk_starts,
        NB=NB, NPC=NPC, NG=NG, S=S, calls=calls, IWTOT=IWTOT, TOTS=TOTS,
        NCALLS=NCALLS, idx_tab=idx_tab, last_of_block=last_of_block,
        n_nodes=n_nodes, NTAB=NTAB,
        block_ts=S.sum(1),
    )


# ----------------------------------------------------------------------------
# bass builders

def build_hext(seg_len):
    """Launch 1: per core computes table payload rows for seg_len nodes.

    inputs : xT [128, seg_len] fp16, Wt [128,128] fp16,
             as_rep [128,128] fp32, ad_rep [128,128] fp32
    output : hx [seg_len, PAY] fp16  rows = [h(128) | es(4) | ed(4)]
    """
    nc = bacc.Bacc("TRN2", num_swdge_queues=4)
    xT = nc.dram_tensor("xT", [128, seg_len], F16, kind="ExternalInput")
    Wt = nc.dram_tensor("Wt", [128, 128], F16, kind="ExternalInput")
    as_rep = nc.dram_tensor("as_rep", [128, 128], F32, kind="ExternalInput")
    ad_rep = nc.dram_tensor("ad_rep", [128, 128], F32, kind="ExternalInput")
    hx = nc.dram_tensor("hx", [seg_len, PAY], F16, kind="ExternalOutput")

    ntiles = (seg_len + 127) // 128
    with tile.TileContext(nc) as tc:
        with (
            tc.tile_pool(name="consts", bufs=1) as cpool,
            tc.tile_pool(name="work", bufs=4) as pool,
            tc.tile_pool(name="ps", bufs=4, space="PSUM") as pp,
        ):
            wt = cpool.tile([128, 128], F16)
            nc.sync.dma_start(out=wt[:], in_=Wt[:])
            asr = cpool.tile([128, 128], F32)
            nc.sync.dma_start(out=asr[:], in_=as_rep[:])
            adr = cpool.tile([128, 128], F32)
            nc.sync.dma_start(out=adr[:], in_=ad_rep[:])
            for t in range(ntiles):
                nt = min(128, seg_len - t * 128)
                xt = pool.tile([128, 128], F16, tag="xt")
                nc.sync.dma_start(out=xt[:, :nt], in_=xT[:, t * 128:t * 128 + nt])
                ph = pp.tile([128, 128], F32)
                nc.tensor.matmul(ph[:nt, :], lhsT=xt[:, :nt], rhs=wt[:],
                                 start=True, stop=True)
                row = pool.tile([128, PAY], F16, tag="row")
                nc.vector.tensor_copy(row[:nt, 0:128], ph[:nt, :])
                scr = pool.tile([128, 32], F32, tag="scr")
                for h in range(H):
                    nc.vector.scalar_tensor_tensor(
                        out=scr[:nt, :], in0=ph[:nt, h * 32:(h + 1) * 32],
                        scalar=1.0, in1=asr[:nt, h * 32:(h + 1) * 32],
                        op0=mybir.AluOpType.mult, op1=mybir.AluOpType.mult,
                        accum_out=row[:nt, 128 + h:129 + h])
                for h in range(H):
                    nc.vector.scalar_tensor_tensor(
                        out=scr[:nt, :], in0=ph[:nt, h * 32:(h + 1) * 32],
                        scalar=1.0, in1=adr[:nt, h * 32:(h + 1) * 32],
                        op0=mybir.AluOpType.mult, op1=mybir.AluOpType.mult,
                        accum_out=row[:nt, 132 + h:133 + h])
                nc.sync.dma_start(out=hx[t * 128:t * 128 + nt, :], in_=row[:nt, :])
    nc.compile()
    _split_waits(nc, max_waits=1)
    return nc


def build_msg(plan, layer2):
    """Launch 2/3: slot-major message passing for one layer on each core.

    inputs : tab [NTAB, ROW] fp16, idxs [128, IWTOT] int16,
             edt_all [128, NB*4] fp16, btile [128,128] fp32,
             ident [128,128] fp16,
             (layer1) W2t [128,128] fp16, a2s_rep/a2d_rep [128,128] fp32
    output : layer1: hx2 [NPC, PAY] fp16 ; layer2: outp [NPC, 128] fp32
    """
    NB, NG, S, calls = plan["NB"], plan["NG"], plan["S"], plan["calls"]
    NPC, IWTOT = plan["NPC"], plan["IWTOT"]
    NTAB = plan["NTAB"]
    cs = plan["chunk_starts"]
    last_of_block = plan["last_of_block"]
    block_ts = plan["block_ts"]

    nc = bacc.Bacc("TRN2", num_swdge_queues=4)
    tab = nc.dram_tensor("tab", [NTAB, ROW], F16, kind="ExternalInput")
    idxs = nc.dram_tensor("idxs", [128, IWTOT], I16, kind="ExternalInput")
    eds = nc.dram_tensor("eds", [128, NB * 4], F16, kind="ExternalInput")
    btile = nc.dram_tensor("btile", [128, 128], F32, kind="ExternalInput")
    identt = nc.dram_tensor("ident", [128, 128], F16, kind="ExternalInput")
    if not layer2:
        W2t = nc.dram_tensor("W2t", [128, 128], F16, kind="ExternalInput")
        a2s = nc.dram_tensor("a2s_rep", [128, 128], F32, kind="ExternalInput")
        a2d = nc.dram_tensor("a2d_rep", [128, 128], F32, kind="ExternalInput")
        hx2 = nc.dram_tensor("hx2", [NPC, PAY], F16, kind="ExternalOutput")
    else:
        outp = nc.dram_tensor("outp", [NPC, 128], F32, kind="ExternalOutput")

    # chunk sizes for gather source windows
    csz = [int((cs[c + 1] if c + 1 < NCHUNKS else NTAB) - cs[c])
           for c in range(NCHUNKS)]

    A = mybir.AluOpType
    AF = mybir.ActivationFunctionType
    qn = 0
    with tile.TileContext(nc) as tc:
        with (
            tc.tile_pool(name="consts", bufs=1) as cpool,
            tc.tile_pool(name="gath", bufs=12) as gp,
            tc.tile_pool(name="wp", bufs=8) as wp,
            tc.tile_pool(name="msgp", bufs=10) as mp,
            tc.tile_pool(name="finp", bufs=3) as fp_,
            tc.tile_pool(name="psb", bufs=6, space="PSUM") as ppb,
            tc.tile_pool(name="psx", bufs=2, space="PSUM") as ppx,
        ):
            ident = cpool.tile([128, 128], F16)
            nc.sync.dma_start(out=ident[:], in_=identt[:])
            bt = cpool.tile([128, 128], F32)
            nc.sync.dma_start(out=bt[:], in_=btile[:])
            edt = cpool.tile([128, NB * 4], F16)
            nc.sync.dma_start(out=edt[:], in_=eds[:])
            itab = cpool.tile([128, IWTOT], I16)
            nc.sync.dma_start(out=itab[:], in_=idxs[:])
            zt = cpool.tile([128, 3 * 132], F16)
            nc.vector.memset(zt[:], 0.0)
            if not layer2:
                w2 = cpool.tile([128, 128], F16)
                nc.sync.dma_start(out=w2[:], in_=W2t[:])
                a2sr = cpool.tile([128, 128], F32)
                nc.sync.dma_start(out=a2sr[:], in_=a2s[:])
                a2dr = cpool.tile([128, 128], F32)
                nc.sync.dma_start(out=a2dr[:], in_=a2d[:])
                # device-side pack: w2e = [W2 | W2@bd(a2s) | W2@bd(a2d)]
                w2e = cpool.tile([128, PAY], F16)
                nc.vector.tensor_copy(w2e[:, 0:128], w2[:])
                pscr = cpool.tile([128, 32], F32)
                for h in range(H):
                    nc.vector.scalar_tensor_tensor(
                        out=pscr[:], in0=w2[:, h * 32:(h + 1) * 32], scalar=1.0,
                        in1=a2sr[:, h * 32:(h + 1) * 32],
                        op0=A.mult, op1=A.mult,
                        accum_out=w2e[:, 128 + h:129 + h])
                for h in range(H):
                    nc.vector.scalar_tensor_tensor(
                        out=pscr[:], in0=w2[:, h * 32:(h + 1) * 32], scalar=1.0,
                        in1=a2dr[:, h * 32:(h + 1) * 32],
                        op0=A.mult, op1=A.mult,
                        accum_out=w2e[:, 132 + h:133 + h])

            pb_of = {}       # open PSUM tile per block
            ci = 0
            for g in range(NG):
                jlo, jhi = g * GRP, min((g + 1) * GRP, NB)
                for j in range(jlo, jhi):
                    if block_ts[j] == 0:
                        continue
                    pb = ppb.tile([128, 3 * 132], F32, tag="pb")
                    pb_of[j] = pb
                    nc.tensor.matmul(pb[:], lhsT=ident[:], rhs=zt[:],
                                     start=True, stop=False)
                while ci < len(calls) and calls[ci]["g"] == g:
                    cl = calls[ci]
                    c, ns, col, segs = cl["c"], cl["ns"], cl["col"], cl["segs"]
                    gt = gp.tile([128, MAXCALL * ROW], F16, tag="gt")
                    nc.gpsimd.dma_gather(
                        gt[:, :ns * ROW].rearrange("p (k e) -> p k e", e=ROW),
                        tab[int(cs[c]):int(cs[c]) + csz[c], :],
                        itab[:, col:col + ns * 8], ns * 128, ns * 128, ROW,
                        single_packet=False, queue_num=qn % 4)
                    qn += 1
                    gv = gt[:, :ns * ROW].rearrange("p (k e) -> p k e", e=ROW)
                    # logits: lg = es + ed (per block segment)
                    wt_ = wp.tile([128, MAXCALL * 4], F32, tag="wt")
                    for (s0, s1, j, q0) in segs:
                        _e = edt[:, j * 4:(j + 1) * 4]
                        nc.vector.tensor_tensor(
                            out=wt_[:, s0 * 4:s1 * 4].rearrange(
                                "p (k e) -> p k e", e=4),
                            in0=gv[:, s0:s1, 128:132],
                            in1=bass.AP(_e.tensor, _e.offset,
                                        [_e.ap[0], [0, s1 - s0], [1, 4]]),
                            op=A.add)
                    # w = exp(lrelu(lg)) ; w lands in msg cols 128:132
                    nc.vector.scalar_tensor_tensor(
                        out=wt_[:, :ns * 4], in0=wt_[:, :ns * 4],
                        scalar=NEG_SLOPE, in1=wt_[:, :ns * 4],
                        op0=A.mult, op1=A.max)
                    msg = mp.tile([128, MAXCALL * 132], F16, tag="msg")
                    msg_v = msg[:, :ns * 132].rearrange("p (k e) -> p k e", e=132)
                    nc.scalar.activation(
                        msg_v[:, :, 128:132],
                        wt_[:, :ns * 4].rearrange("p (k e) -> p k e", e=4),
                        AF.Exp)
                    # msg = h * w
                    wv = msg_v[:, :, 128:132]
                    nc.vector.tensor_tensor(
                        out=msg_v[:, :, 0:128].rearrange(
                            "p k (h d) -> p k h d", d=32),
                        in0=gv[:, :, 0:128].rearrange("p k (h d) -> p k h d", d=32),
                        in1=bass.AP(wv.tensor, wv.offset,
                                    [wv.ap[0], [132, ns], [1, 4], [0, 32]]),
                        op=A.mult)
                    # 3-stripe accumulating matmuls per segment
                    for si_, (s0, s1, j, q0) in enumerate(segs):
                        pb = pb_of[j]
                        is_last_seg = last_of_block[j] == (ci, si_)
                        t0 = s0
                        while t0 < s1:
                            te = min(t0 + 3, s1)
                            stop = is_last_seg and te == s1
                            nc.tensor.matmul(
                                pb[:, :(te - t0) * 132], lhsT=ident[:],
                                rhs=msg[:, t0 * 132:te * 132],
                                start=False, stop=stop)
                            t0 = te
                    ci += 1
                # finalize blocks of this group
                for j in range(jlo, jhi):
                    if block_ts[j] == 0:
                        continue
                    pb = pb_of.pop(j)
                    acc = fp_.tile([128, 3 * 132], F32, tag="acc")
                    nc.scalar.activation(acc[:], pb[:], AF.Identity)
                    nc.vector.tensor_tensor(out=acc[:, 0:132], in0=acc[:, 0:132],
                                            in1=acc[:, 132:264], op=A.add)
                    nc.vector.tensor_tensor(out=acc[:, 0:132], in0=acc[:, 0:132],
                                            in1=acc[:, 264:396], op=A.add)
                    den = fp_.tile([128, 4], F32, tag="den")
                    nc.vector.tensor_scalar_add(den[:], acc[:, 128:132], 1e-20)
                    nc.vector.reciprocal(den[:], den[:])
                    t1 = fp_.tile([128, 128], F32, tag="t1")
                    nc.vector.tensor_tensor(
                        out=t1[:].rearrange("p (h d) -> p h d", d=32),
                        in0=acc[:, 0:128].rearrange("p (h d) -> p h d", d=32),
                        in1=bass.AP(den.tensor, den.offset,
                                    [den.ap[0], [1, 4], [0, 32]]),
                        op=A.mult)
                    nc.vector.tensor_tensor(out=t1[:], in0=t1[:], in1=bt[:],
                                            op=A.add)
                    if not layer2:
                        x2 = fp_.tile([128, 128], F16, tag="x2")
                        nc.scalar.activation(x2[:], t1[:], AF.Relu)
                        px = ppx.tile([128, 128], F16, tag="tx")
                        nc.tensor.transpose(px[:], x2[:], ident[:])
                        x2t = fp_.tile([128, 128], F16, tag="x2t")
                        nc.scalar.activation(x2t[:], px[:], AF.Identity)
                        ph2 = ppx.tile([128, PAY], F32, tag="tx")
                        nc.tensor.matmul(ph2[:], lhsT=x2t[:], rhs=w2e[:],
                                         start=True, stop=True)
                        row = fp_.tile([128, PAY], F16, tag="row")
                        nc.scalar.activation(row[:], ph2[:], AF.Identity)
                        nc.sync.dma_start(out=hx2[j * 128:(j + 1) * 128, :],
                                          in_=row[:])
                    else:
                        et = fp_.tile([128, 128], F32, tag="et")
                        nc.scalar.activation(et[:], t1[:], AF.Exp)
                        ssum = fp_.tile([128, 1], F32, tag="ssum")
                        nc.vector.tensor_reduce(ssum[:], et[:],
                                                axis=mybir.AxisListType.X,
                                                op=A.add)
                        nc.scalar.activation(ssum[:], ssum[:], AF.Ln)
                        nc.vector.tensor_scalar_mul(ssum[:], ssum[:], -1.0)
                        to = fp_.tile([128, 128], F32, tag="to")
                        nc.scalar.activation(to[:], t1[:], AF.Identity,
                                             bias=ssum[:, 0:1])
                        nc.sync.dma_start(out=outp[j * 128:(j + 1) * 128, :],
                                          in_=to[:])
    nc.compile()
    _split_waits(nc, max_waits=1)
    return nc


# ----------------------------------------------------------------------------
# runner

def _rep_heads(a):
    return np.tile(np.asarray(a).reshape(1, -1).astype(np.float32), (128, 1))


def _run(nc, in_maps):
    from concourse.bass_utils import run_bass_kernel_spmd
    return run_bass_kernel_spmd(nc, in_maps, core_ids=list(range(NCORES)),
                                trace=False).results


def _assemble_tab(plan, hx_by_core, core_seg_nodes):
    """hx rows (per-core, perm order) -> full table with sentinels."""
    n = plan["n_nodes"]
    tab = np.zeros((plan["NTAB"], ROW), np.float16)
    tab[plan["chunk_starts"], 128:132] = SENT
    tpos = plan["tpos"]
    for core in range(NCORES):
        nodes = core_seg_nodes[core]
        tab[tpos[nodes], :PAY] = hx_by_core[core]
    return tab


def _tab_from_blocks(plan, hx2_by_core):
    """hx2 rows (per-core, block order) -> full table with sentinels."""
    tab = np.zeros((plan["NTAB"], ROW), np.float16)
    tab[plan["chunk_starts"], 128:132] = SENT
    tpos = plan["tpos"]
    core_nodes = plan["core_nodes"]
    for core in range(NCORES):
        cn = core_nodes[core]
        vm = cn >= 0
        tab[tpos[cn[vm]], :PAY] = np.asarray(hx2_by_core[core])[vm]
    return tab


def _eds_of(plan, tab):
    """per-core [128, NB*4] fp16 ed table in (part, block*4) layout."""
    NB = plan["NB"]
    core_nodes = plan["core_nodes"]
    tpos = plan["tpos"]
    eds = np.zeros((NCORES, 128, NB * 4), np.float16)
    for core in range(NCORES):
        cn = core_nodes[core].reshape(NB, 128)     # [j, p]
        vm = cn >= 0
        vals = np.zeros((NB, 128, 4), np.float16)
        vals[vm] = tab[tpos[cn[vm]], 132:136]
        eds[core] = vals.transpose(1, 0, 2).reshape(128, NB * 4)
    return eds


def run_pipeline(inputs, n_nodes, run=_run):
    edge = np.asarray(inputs["edge"])
    x = np.asarray(inputs["features"], np.float32)
    W1 = np.asarray(inputs["W1"], np.float32)
    a1s = np.asarray(inputs["a1_src"], np.float32)
    a1d = np.asarray(inputs["a1_dst"], np.float32)
    b1 = np.asarray(inputs["b1"], np.float32)
    W2 = np.asarray(inputs["W2"], np.float32)
    a2s = np.asarray(inputs["a2_src"], np.float32)
    a2d = np.asarray(inputs["a2_dst"], np.float32)
    b2 = np.asarray(inputs["b2"], np.float32)

    plan = build_plan(edge, n_nodes)
    NB, NPC = plan["NB"], plan["NPC"]
    core_nodes = plan["core_nodes"]

    # ---- launch 1: hext1 over nodes in table order split across cores
    tord = np.argsort(plan["tpos"])          # nodes in table-row order
    seg = (n_nodes + NCORES - 1) // NCORES
    pad = seg * NCORES - n_nodes
    tord_p = np.concatenate([tord, tord[:pad]]) if pad else tord
    nc1 = build_hext(seg)
    in1, seg_nodes_of = [], []
    for core in range(NCORES):
        seg_nodes = tord_p[core * seg:(core + 1) * seg]
        seg_nodes_of.append(seg_nodes)
        xT = np.ascontiguousarray(x[seg_nodes].astype(np.float16).T)
        in1.append({
            "xT": xT, "Wt": W1.astype(np.float16),
            "as_rep": _rep_heads(a1s), "ad_rep": _rep_heads(a1d),
        })
    res1 = run(nc1, in1)
    hx_by_core = [np.asarray(res1[c]["hx"]) for c in range(NCORES)]
    # dedupe the pad overlap: later writes win, identical rows anyway
    tab1 = _assemble_tab(plan, hx_by_core, seg_nodes_of)
    eds1 = _eds_of(plan, tab1)

    ident = np.eye(128, dtype=np.float16)

    # ---- launch 2: layer-1 message passing + inline h2 table rows
    nc2 = build_msg(plan, layer2=False)
    in2 = []
    for core in range(NCORES):
        in2.append({
            "tab": tab1, "idxs": plan["idx_tab"][core],
            "eds": eds1[core],
            "btile": np.tile(b1.reshape(1, -1), (128, 1)).astype(np.float32),
            "ident": ident, "W2t": W2.astype(np.float16),
            "a2s_rep": _rep_heads(a2s), "a2d_rep": _rep_heads(a2d),
        })
    res2 = run(nc2, in2)
    tab2 = _tab_from_blocks(plan, [res2[c]["hx2"] for c in range(NCORES)])
    eds2 = _eds_of(plan, tab2)

    # ---- launch 3: layer-2 message passing + log_softmax
    nc3 = build_msg(plan, layer2=True)
    in3 = []
    for core in range(NCORES):
        in3.append({
            "tab": tab2, "idxs": plan["idx_tab"][core],
            "eds": eds2[core],
            "btile": np.tile(b2.reshape(1, -1), (128, 1)).astype(np.float32),
            "ident": ident,
        })
    res3 = run(nc3, in3)

    out = np.zeros((n_nodes, H * D_OUT), np.float32)
    for core in range(NCORES):
        cn = core_nodes[core]
        vm = cn >= 0
        out[cn[vm]] = np.asarray(res3[core]["outp"])[vm]
    return out


def kernel(**inputs):
    return run_pipeline(inputs, N_NODES).astype(np.float32)


# revision 4
# speedup vs baseline: 1.1466x; 1.0234x over previous
"""Trainium2 Bass kernel v2 for 2-layer GAT (nn_GAT_62182536511740).

Strategy (slot-major message passing, v2):
  * Host (pure indexing): greedy chunk balancing (each dst node's in-edges
    split evenly across 4 table chunks), degree-sorted block clustering,
    multi-block gather calls (4-block groups per chunk) to amortize SWDGE
    fixed overhead.  No edge dedup; padded slots gather a per-chunk sentinel
    row (h=0, es=-3e4) so their softmax weight is exactly 0.
  * 3 launches on 8 cores (SPMD):
      1) hext1: h1 = x@W1 plus per-node attention logits -> 272B row payload
      2) msg layer 1 (+ inline h2-table build)
      3) msg layer 2 (+ log_softmax)
  * Gathers: 4-queue dma_gather (int16 idx preloaded in ONE dma), 512B rows
    [h(128)|es(4)|ed(4)|junk] fp16.
  * Per call: vector es+ed add (per block segment), scalar Lrelu+Exp (w lands
    in msg cols 128:132), vector h*w; 3-stripe identity matmuls accumulate
    into a [128,396] PSUM bank per block (folded at finalize).
"""
import numpy as np

import concourse.bacc as bacc
import concourse.bass as bass
import concourse.mybir as mybir
import concourse.tile as tile
from concourse.vector_clock import ScopedClock

# ----------------------------------------------------------------------------
N_NODES = 100000
N_EDGES = 1600000
D_IN = 128
H = 4
D_HID = 32
D_OUT = 32
NEG_SLOPE = 0.2

NCORES = 8
NCHUNKS = 4
ROW = 256          # fp16 elements per table row (512 B)
PAY = 136          # payload elements per row [h(128)|es(4)|ed(4)]
MAXCALL = 8        # stripes per dma_gather call (<=1024 idxs, ucode ring cap)
GRP = 4            # blocks per call-group
SENT = -30000.0    # sentinel es value
F32 = mybir.dt.float32
F16 = mybir.dt.float16
I16 = mybir.dt.int16

# ----------------------------------------------------------------------------
# toolchain workaround: walrus rejects instructions with many sync waits.


def _split_waits(nc, max_waits=1):
    for bb in nc.main_func.blocks:
        insts = bb.instructions
        i = 0
        while i < len(insts):
            ins = insts[i]
            si = ins.sync_info
            if si is not None and si.on_wait and len(si.on_wait) > max_waits:
                waits = list(si.on_wait)
                keep = waits[-max_waits:]
                move = waits[: len(waits) - len(keep)]
                del si.on_wait[:]
                si.on_wait.extend(keep)
                new_nops = []
                for w in move:
                    nop = nc.engines[ins.engine].nop(nofuse=True)
                    nop_ins = nop.ins
                    emitted = nc.cur_bb.bb.instructions
                    assert emitted[-1] is nop_ins
                    emitted.pop()
                    if nop_ins.sync_info is None:
                        nop_ins.sync_info = mybir.SyncInfo(on_wait=[w], on_update=[])
                    else:
                        nop_ins.sync_info.on_wait.append(w)
                    new_nops.append(nop_ins)
                insts[i:i] = new_nops
                i += len(new_nops)
            i += 1


def _drain_and_barrier_split(self, tick_clock, wait_clock):
    nc = self.nc
    drain_inst = nc.sync.drain()
    wait_clock.add_sem_waits(
        drain_inst.ins, ScopedClock({None: tick_clock.global_clock})
    )
    si = drain_inst.ins.sync_info
    if si is not None and si.on_wait and len(si.on_wait) > 1:
        waits = list(si.on_wait)
        del si.on_wait[:]
        bb = nc.cur_bb.bb
        assert bb.instructions[-1] is drain_inst.ins
        bb.instructions.pop()
        for w in waits:
            nop = nc.sync.nop(nofuse=True)
            nsi = nop.ins.sync_info
            if nsi is None:
                nop.ins.sync_info = mybir.SyncInfo(on_wait=[w], on_update=[])
            else:
                nsi.on_wait.append(w)
        bb.instructions.append(drain_inst.ins)
    nc.all_engine_barrier()
    assert self.sems is not None
    popped = nc._tile_sem_poison_stack.pop()
    assert popped is self._sem_poison
    nc.clear_and_free_semaphores(list(self.sems.allocated().values()))
    nc.all_engine_barrier()


tile.TileContext._drain_and_barrier = _drain_and_barrier_split


# ----------------------------------------------------------------------------
# host planning (pure indexing)

def _balance_chunks(src, dst, n_nodes):
    """Greedy + refinement: assign each node (as source) to a chunk, keeping
    every dst's per-chunk in-edge counts flat.  Returns chunk_map, counts."""
    # CSR by source
    order = np.argsort(src, kind="stable")
    d_sorted = dst[order]
    starts = np.searchsorted(src[order], np.arange(n_nodes + 1))
    outdeg = np.diff(starts)
    q = np.bincount(dst, minlength=n_nodes).astype(np.float64) / NCHUNKS

    capmax = 32000
    cnt = np.zeros((n_nodes, NCHUNKS), np.int32)
    cap = np.zeros(NCHUNKS, np.int64)
    chunk_map = np.zeros(n_nodes, np.int64)
    proc = np.argsort(-outdeg, kind="stable")
    for s in proc:
        lo, hi = starts[s], starts[s + 1]
        if hi == lo:
            c = int(np.argmin(cap))
        else:
            ds = d_sorted[lo:hi]
            pen = (4.0 ** np.clip(cnt[ds] + 1 - q[ds][:, None], -8, 8)).sum(0)
            pen[cap >= capmax] = 1e18
            c = int(np.argmin(pen))
            cnt[ds, c] += 1
        chunk_map[s] = c
        cap[c] += 1

    # local-search refinement: move sources out of overfull cells
    for _ in range(3):
        moved = 0
        for s in range(n_nodes):
            lo, hi = starts[s], starts[s + 1]
            if hi == lo:
                continue
            ds = d_sorted[lo:hi]
            c0 = int(chunk_map[s])
            base = 4.0 ** np.clip(cnt[ds] - q[ds][:, None], -8, 8)
            gain = base[:, c0].sum() * 0.75
            add = base.sum(0) * 3.0
            add[c0] = 1e18
            add[cap >= capmax] = 1e18
            cb = int(np.argmin(add))
            if add[cb] < gain - 1e-12:
                cnt[ds, c0] -= 1
                cnt[ds, cb] += 1
                cap[c0] -= 1
                cap[cb] += 1
                chunk_map[s] = cb
                moved += 1
        if moved < n_nodes // 200:
            break
    return chunk_map, cnt


def build_plan(edge, n_nodes):
    src = np.asarray(edge[0], np.int64)
    dst = np.asarray(edge[1], np.int64)
    E = len(src)

    chunk_map, prof = _balance_chunks(src, dst, n_nodes)

    # block clustering: group nodes by (max count, argmax chunk, profile)
    smax = prof.max(1)
    amax = prof.argmax(1)
    order = np.lexsort((prof[:, 3], prof[:, 2], prof[:, 1], prof[:, 0],
                        amax, smax))

    nblk_tot = (n_nodes + 127) // 128
    NB = (nblk_tot + NCORES - 1) // NCORES
    NPC = NB * 128
    core_nodes = -np.ones((NCORES, NPC), np.int64)
    bi = 0
    for j in range(NB):
        for c_ in range(NCORES):
            core = c_ if (j % 2 == 0) else (NCORES - 1 - c_)
            if bi >= nblk_tot:
                continue
            blk = order[bi * 128:(bi + 1) * 128]
            core_nodes[core, j * 128:j * 128 + len(blk)] = blk
            bi += 1

    # table order: chunk-major (sentinel first in each chunk), then
    # (core, block, partition)
    tpos = -np.ones(n_nodes, np.int64)       # table row per node
    loc = -np.ones(n_nodes, np.int64)        # chunk-local index (>=1)
    chunk_starts = np.zeros(NCHUNKS, np.int64)
    t = 0
    for c in range(NCHUNKS):
        chunk_starts[c] = t
        t += 1                               # sentinel row
        lt = 1
        for core in range(NCORES):
            for j in range(NB):
                blk = core_nodes[core, j * 128:(j + 1) * 128]
                sel = blk[blk >= 0]
                sel = sel[chunk_map[sel] == c]
                tpos[sel] = t + np.arange(len(sel))
                loc[sel] = lt + np.arange(len(sel))
                t += len(sel)
                lt += len(sel)
        assert lt <= 32767
    NTAB = t
    assert NTAB == n_nodes + NCHUNKS

    node_core = -np.ones(n_nodes, np.int64)
    node_blk = -np.ones(n_nodes, np.int64)
    node_part = -np.ones(n_nodes, np.int64)
    for core in range(NCORES):
        cn = core_nodes[core]
        pos = np.nonzero(cn >= 0)[0]
        node_core[cn[pos]] = core
        node_blk[cn[pos]] = pos // 128
        node_part[cn[pos]] = pos % 128

    ecore = node_core[dst]
    eblk = node_blk[dst]
    epart = node_part[dst]
    echunk = chunk_map[src]

    # slot index within (core, blk, chunk, part)
    gkey = ((ecore * NB + eblk) * NCHUNKS + echunk) * 128 + epart
    eorder = np.lexsort((gkey,))
    gk_sorted = gkey[eorder]
    grp_start = np.r_[True, gk_sorted[1:] != gk_sorted[:-1]]
    idx_in_grp = np.arange(E) - np.maximum.accumulate(
        np.where(grp_start, np.arange(E), 0))
    eslot = np.empty(E, np.int64)
    eslot[eorder] = idx_in_grp

    cnt = np.zeros((NCORES, NB, NCHUNKS, 128), np.int32)
    np.add.at(cnt, (ecore, eblk, echunk, epart), 1)
    S = cnt.max(axis=(0, 3)).astype(np.int64)        # [NB, NCHUNKS] compiled
    TOTS = int(S.sum())

    # ---- static call schedule: 4-block groups, calls per (group, chunk)
    # stripes of chunk c within group g stream across its blocks.
    NG = (NB + GRP - 1) // GRP
    # stripe base of (j, c) within its group-chunk stream
    qbase = np.zeros((NB, NCHUNKS), np.int64)
    for g in range(NG):
        jlo, jhi = g * GRP, min((g + 1) * GRP, NB)
        for c in range(NCHUNKS):
            acc = 0
            for j in range(jlo, jhi):
                qbase[j, c] = acc
                acc += int(S[j, c])

    # calls: list of dicts (g, c, ns, col, segs=[(s0, s1, j, q0)])
    calls = []
    col = 0
    for g in range(NG):
        jlo, jhi = g * GRP, min((g + 1) * GRP, NB)
        for c in range(NCHUNKS):
            ts = sum(int(S[j, c]) for j in range(jlo, jhi))
            k = 0
            while k < ts:
                ns = min(MAXCALL, ts - k)
                # segments of this call
                segs = []
                for j in range(jlo, jhi):
                    b0, b1 = int(qbase[j, c]), int(qbase[j, c] + S[j, c])
                    s0 = max(b0, k) - k
                    s1 = min(b1, k + ns) - k
                    if s1 > s0:
                        segs.append((s0, s1, j, max(b0, k) - b0))
                calls.append(dict(g=g, c=c, ns=ns, col=col, segs=segs))
                col += ns * 8
                k += ns
    IWTOT = col
    NCALLS = len(calls)

    # per-block last-matmul bookkeeping: last (call index, seg index) per block
    last_of_block = {}
    for ci, cl in enumerate(calls):
        for si_, (s0, s1, j, q0) in enumerate(cl["segs"]):
            last_of_block[j] = (ci, si_)

    # ---- per-core idx tables [128, IWTOT] int16 (16-wrap, x8 replicated)
    # call-position value: slot (stripe k within call, part p) at col k*128+p
    idx_tab = np.zeros((NCORES, 128, IWTOT), np.int16)
    # map each edge to (core, call col position)
    # stripe within group-chunk stream = qbase[j,c] + eslot
    estripe = qbase[eblk, echunk] + eslot
    # call index within (g, c): precompute per (g,c) col bases & stripe starts
    callmeta = {}
    for ci, cl in enumerate(calls):
        callmeta.setdefault((cl["g"], cl["c"]), []).append(ci)
    # for vector lookup: per (g, c), stripes split in groups of 8
    egrp = eblk // GRP
    ecall_k = estripe // MAXCALL      # which call within (g,c) stream
    ecall_s = estripe % MAXCALL       # stripe within call
    # col base per (g, c, k)
    colbase = {}
    for (g, c), cis in callmeta.items():
        for k, ci in enumerate(cis):
            colbase[(g, c, k)] = calls[ci]["col"]
    ecol = np.fromiter(
        (colbase[(int(g_), int(c_), int(k_))] for g_, c_, k_ in
         zip(egrp, echunk, ecall_k)),
        np.int64, count=E)
    # position within call = stripe*128 + part ; value = chunk-local row
    epos = ecall_s * 128 + epart
    # flat per-core fill
    for core in range(NCORES):
        esel = np.nonzero(ecore == core)[0]
        flat = np.zeros((IWTOT // 8) * 128, np.int64)  # positions per call run
        # column in 16-wrap layout: call col + (pos // 16) ... build via
        # full flat position: fpos = ecol*16 + epos  (each col covers 16 pos)
        fpos = ecol[esel] * 16 + epos[esel]
        vals = loc[src[esel]]
        flat_full = np.zeros(IWTOT * 16, np.int64)
        flat_full[fpos] = vals
        wrap = flat_full.reshape(IWTOT, 16).T.astype(np.int16)  # [16, IWTOT]
        idx_tab[core] = np.tile(wrap, (8, 1))

    return dict(
        tpos=tpos, loc=loc, core_nodes=core_nodes, chunk_starts=chunk_starts,
        NB=NB, NPC=NPC, NG=NG, S=S, calls=calls, IWTOT=IWTOT, TOTS=TOTS,
        NCALLS=NCALLS, idx_tab=idx_tab, last_of_block=last_of_block,
        n_nodes=n_nodes, NTAB=NTAB,
        block_ts=S.sum(1),
    )


# ----------------------------------------------------------------------------
# bass builders

def build_hext(seg_len):
    """Launch 1: per core computes table payload rows for seg_len nodes.

    inputs : xT [128, seg_len] fp16, Wt [128,128] fp16,
             as_rep [128,128] fp32, ad_rep [128,128] fp32
    output : hx [seg_len, PAY] fp16  rows = [h(128) | es(4) | ed(4)]
    """
    nc = bacc.Bacc("TRN2", num_swdge_queues=4)
    xT = nc.dram_tensor("xT", [128, seg_len], F16, kind="ExternalInput")
    Wt = nc.dram_tensor("Wt", [128, 128], F16, kind="ExternalInput")
    as_rep = nc.dram_tensor("as_rep", [128, 128], F32, kind="ExternalInput")
    ad_rep = nc.dram_tensor("ad_rep", [128, 128], F32, kind="ExternalInput")
    hx = nc.dram_tensor("hx", [seg_len, PAY], F16, kind="ExternalOutput")

    ntiles = (seg_len + 127) // 128
    with tile.TileContext(nc) as tc:
        with (
            tc.tile_pool(name="consts", bufs=1) as cpool,
            tc.tile_pool(name="work", bufs=6) as pool,
            tc.tile_pool(name="ps", bufs=4, space="PSUM") as pp,
        ):
            wt = cpool.tile([128, 128], F16)
            nc.sync.dma_start(out=wt[:], in_=Wt[:])
            asr = cpool.tile([128, 128], F32)
            nc.sync.dma_start(out=asr[:], in_=as_rep[:])
            adr = cpool.tile([128, 128], F32)
            nc.sync.dma_start(out=adr[:], in_=ad_rep[:])
            xall = cpool.tile([128, seg_len], F16)
            nc.sync.dma_start(out=xall[:], in_=xT[:])
            for t in range(ntiles):
                nt = min(128, seg_len - t * 128)
                xt = xall[:, t * 128:t * 128 + nt]
                ph = pp.tile([128, 128], F32)
                nc.tensor.matmul(ph[:nt, :], lhsT=xt, rhs=wt[:],
                                 start=True, stop=True)
                row = pool.tile([128, PAY], F16, tag="row")
                nc.scalar.activation(row[:nt, 0:128], ph[:nt, :],
                                     mybir.ActivationFunctionType.Identity)
                scr = pool.tile([128, 32], F32, tag="scr")
                for h in range(H):
                    nc.vector.scalar_tensor_tensor(
                        out=scr[:nt, :], in0=ph[:nt, h * 32:(h + 1) * 32],
                        scalar=1.0, in1=asr[:nt, h * 32:(h + 1) * 32],
                        op0=mybir.AluOpType.mult, op1=mybir.AluOpType.mult,
                        accum_out=row[:nt, 128 + h:129 + h])
                for h in range(H):
                    nc.vector.scalar_tensor_tensor(
                        out=scr[:nt, :], in0=ph[:nt, h * 32:(h + 1) * 32],
                        scalar=1.0, in1=adr[:nt, h * 32:(h + 1) * 32],
                        op0=mybir.AluOpType.mult, op1=mybir.AluOpType.mult,
                        accum_out=row[:nt, 132 + h:133 + h])
                nc.sync.dma_start(out=hx[t * 128:t * 128 + nt, :], in_=row[:nt, :])
    nc.compile()
    _split_waits(nc, max_waits=1)
    return nc


def build_msg(plan, layer2):
    """Launch 2/3: slot-major message passing for one layer on each core.

    inputs : tab [NTAB, ROW] fp16, idxs [128, IWTOT] int16,
             edt_all [128, NB*4] fp16, btile [128,128] fp32,
             ident [128,128] fp16,
             (layer1) W2t [128,128] fp16, a2s_rep/a2d_rep [128,128] fp32
    output : layer1: hx2 [NPC, PAY] fp16 ; layer2: outp [NPC, 128] fp32
    """
    NB, NG, S, calls = plan["NB"], plan["NG"], plan["S"], plan["calls"]
    NPC, IWTOT = plan["NPC"], plan["IWTOT"]
    NTAB = plan["NTAB"]
    cs = plan["chunk_starts"]
    last_of_block = plan["last_of_block"]
    block_ts = plan["block_ts"]

    nc = bacc.Bacc("TRN2", num_swdge_queues=4)
    tab = nc.dram_tensor("tab", [NTAB, ROW], F16, kind="ExternalInput")
    idxs = nc.dram_tensor("idxs", [128, IWTOT], I16, kind="ExternalInput")
    eds = nc.dram_tensor("eds", [128, NB * 4], F16, kind="ExternalInput")
    btile = nc.dram_tensor("btile", [128, 128], F32, kind="ExternalInput")
    identt = nc.dram_tensor("ident", [128, 128], F16, kind="ExternalInput")
    if not layer2:
        W2t = nc.dram_tensor("W2t", [128, 128], F16, kind="ExternalInput")
        a2s = nc.dram_tensor("a2s_rep", [128, 128], F32, kind="ExternalInput")
        a2d = nc.dram_tensor("a2d_rep", [128, 128], F32, kind="ExternalInput")
        hx2 = nc.dram_tensor("hx2", [NPC, PAY], F16, kind="ExternalOutput")
    else:
        outp = nc.dram_tensor("outp", [NPC, 128], F32, kind="ExternalOutput")

    # chunk sizes for gather source windows
    csz = [int((cs[c + 1] if c + 1 < NCHUNKS else NTAB) - cs[c])
           for c in range(NCHUNKS)]

    A = mybir.AluOpType
    AF = mybir.ActivationFunctionType
    qn = 0
    with tile.TileContext(nc) as tc:
        with (
            tc.tile_pool(name="consts", bufs=1) as cpool,
            tc.tile_pool(name="gath", bufs=12) as gp,
            tc.tile_pool(name="wp", bufs=8) as wp,
            tc.tile_pool(name="msgp", bufs=10) as mp,
            tc.tile_pool(name="finp", bufs=3) as fp_,
            tc.tile_pool(name="psb", bufs=6, space="PSUM") as ppb,
            tc.tile_pool(name="psx", bufs=2, space="PSUM") as ppx,
        ):
            ident = cpool.tile([128, 128], F16)
            nc.sync.dma_start(out=ident[:], in_=identt[:])
            bt = cpool.tile([128, 128], F32)
            nc.sync.dma_start(out=bt[:], in_=btile[:])
            edt = cpool.tile([128, NB * 4], F16)
            nc.sync.dma_start(out=edt[:], in_=eds[:])
            itab = cpool.tile([128, IWTOT], I16)
            nc.sync.dma_start(out=itab[:], in_=idxs[:])
            zt = cpool.tile([128, 3 * 132], F16)
            nc.vector.memset(zt[:], 0.0)
            if layer2:
                t1buf = cpool.tile([128, NB * 128], F16)
                nc.vector.memset(t1buf[:], 0.0)
                sumbuf = cpool.tile([128, NB], F32)
                nc.vector.memset(sumbuf[:], 1.0)
            if not layer2:
                w2 = cpool.tile([128, 128], F16)
                nc.sync.dma_start(out=w2[:], in_=W2t[:])
                a2sr = cpool.tile([128, 128], F32)
                nc.sync.dma_start(out=a2sr[:], in_=a2s[:])
                a2dr = cpool.tile([128, 128], F32)
                nc.sync.dma_start(out=a2dr[:], in_=a2d[:])
                # device-side pack: w2e = [W2 | W2@bd(a2s) | W2@bd(a2d)]
                w2e = cpool.tile([128, PAY], F16)
                nc.vector.tensor_copy(w2e[:, 0:128], w2[:])
                pscr = cpool.tile([128, 32], F32)
                for h in range(H):
                    nc.vector.scalar_tensor_tensor(
                        out=pscr[:], in0=w2[:, h * 32:(h + 1) * 32], scalar=1.0,
                        in1=a2sr[:, h * 32:(h + 1) * 32],
                        op0=A.mult, op1=A.mult,
                        accum_out=w2e[:, 128 + h:129 + h])
                for h in range(H):
                    nc.vector.scalar_tensor_tensor(
                        out=pscr[:], in0=w2[:, h * 32:(h + 1) * 32], scalar=1.0,
                        in1=a2dr[:, h * 32:(h + 1) * 32],
                        op0=A.mult, op1=A.mult,
                        accum_out=w2e[:, 132 + h:133 + h])

            pb_of = {}       # open PSUM tile per block
            ci = 0
            for g in range(NG):
                jlo, jhi = g * GRP, min((g + 1) * GRP, NB)
                for j in range(jlo, jhi):
                    if block_ts[j] == 0:
                        continue
                    pb = ppb.tile([128, 3 * 132], F32, tag="pb")
                    pb_of[j] = pb
                    nc.tensor.matmul(pb[:], lhsT=ident[:], rhs=zt[:],
                                     start=True, stop=False)
                while ci < len(calls) and calls[ci]["g"] == g:
                    cl = calls[ci]
                    c, ns, col, segs = cl["c"], cl["ns"], cl["col"], cl["segs"]
                    gt = gp.tile([128, MAXCALL * ROW], F16, tag="gt")
                    nc.gpsimd.dma_gather(
                        gt[:, :ns * ROW].rearrange("p (k e) -> p k e", e=ROW),
                        tab[int(cs[c]):int(cs[c]) + csz[c], :],
                        itab[:, col:col + ns * 8], ns * 128, ns * 128, ROW,
                        single_packet=False, queue_num=qn % 4)
                    qn += 1
                    gv = gt[:, :ns * ROW].rearrange("p (k e) -> p k e", e=ROW)
                    # logits: lg = es + ed (per block segment)
                    wt_ = wp.tile([128, MAXCALL * 4], F32, tag="wt")
                    for (s0, s1, j, q0) in segs:
                        _e = edt[:, j * 4:(j + 1) * 4]
                        nc.vector.tensor_tensor(
                            out=wt_[:, s0 * 4:s1 * 4].rearrange(
                                "p (k e) -> p k e", e=4),
                            in0=gv[:, s0:s1, 128:132],
                            in1=bass.AP(_e.tensor, _e.offset,
                                        [_e.ap[0], [0, s1 - s0], [1, 4]]),
                            op=A.add)
                    # w = exp(lrelu(lg)) ; w lands in msg cols 128:132
                    nc.vector.scalar_tensor_tensor(
                        out=wt_[:, :ns * 4], in0=wt_[:, :ns * 4],
                        scalar=NEG_SLOPE, in1=wt_[:, :ns * 4],
                        op0=A.mult, op1=A.max)
                    msg = mp.tile([128, MAXCALL * 132], F16, tag="msg")
                    msg_v = msg[:, :ns * 132].rearrange("p (k e) -> p k e", e=132)
                    nc.scalar.activation(
                        msg_v[:, :, 128:132],
                        wt_[:, :ns * 4].rearrange("p (k e) -> p k e", e=4),
                        AF.Exp)
                    # msg = h * w
                    wv = msg_v[:, :, 128:132]
                    nc.vector.tensor_tensor(
                        out=msg_v[:, :, 0:128].rearrange(
                            "p k (h d) -> p k h d", d=32),
                        in0=gv[:, :, 0:128].rearrange("p k (h d) -> p k h d", d=32),
                        in1=bass.AP(wv.tensor, wv.offset,
                                    [wv.ap[0], [132, ns], [1, 4], [0, 32]]),
                        op=A.mult)
                    # 3-stripe accumulating matmuls per segment
                    for si_, (s0, s1, j, q0) in enumerate(segs):
                        pb = pb_of[j]
                        is_last_seg = last_of_block[j] == (ci, si_)
                        t0 = s0
                        while t0 < s1:
                            te = min(t0 + 3, s1)
                            stop = is_last_seg and te == s1
                            nc.tensor.matmul(
                                pb[:, :(te - t0) * 132], lhsT=ident[:],
                                rhs=msg[:, t0 * 132:te * 132],
                                start=False, stop=stop)
                            t0 = te
                    ci += 1
                # finalize blocks of this group (batched)
                live = [j for j in range(jlo, jhi) if block_ts[j] > 0]
                W = len(live)
                if W == 0:
                    continue
                # batched slices assume live blocks are contiguous from jlo
                assert live == list(range(jlo, jlo + W)), live
                t1g = fp_.tile([128, GRP * 128], F32, tag="t1g")
                for k, j in enumerate(live):
                    pb = pb_of.pop(j)
                    acc = fp_.tile([128, 3 * 132], F32, tag="acc")
                    nc.scalar.activation(acc[:], pb[:], AF.Identity)
                    nc.vector.tensor_tensor(out=acc[:, 0:132], in0=acc[:, 0:132],
                                            in1=acc[:, 132:264], op=A.add)
                    nc.vector.tensor_tensor(out=acc[:, 0:132], in0=acc[:, 0:132],
                                            in1=acc[:, 264:396], op=A.add)
                    den = fp_.tile([128, 4], F32, tag="den")
                    nc.vector.tensor_scalar_add(den[:], acc[:, 128:132], 1e-20)
                    nc.vector.reciprocal(den[:], den[:])
                    t1 = t1g[:, k * 128:(k + 1) * 128]
                    nc.vector.tensor_tensor(
                        out=t1.rearrange("p (h d) -> p h d", d=32),
                        in0=acc[:, 0:128].rearrange("p (h d) -> p h d", d=32),
                        in1=bass.AP(den.tensor, den.offset,
                                    [den.ap[0], [1, 4], [0, 32]]),
                        op=A.mult)
                if not layer2:
                    nc.vector.tensor_tensor(
                        out=t1g[:, :W * 128].rearrange("p (k e) -> p k e", e=128),
                        in0=t1g[:, :W * 128].rearrange("p (k e) -> p k e", e=128),
                        in1=bass.AP(bt.tensor, bt.offset,
                                    [bt.ap[0], [0, W], [1, 128]]),
                        op=A.add)
                    x2g = fp_.tile([128, GRP * 128], F16, tag="x2g")
                    nc.scalar.activation(x2g[:, :W * 128], t1g[:, :W * 128],
                                         AF.Relu)
                    pxg = ppx.tile([128, GRP * 128], F16, tag="tx")
                    for k in range(W):
                        nc.tensor.transpose(pxg[:, k * 128:(k + 1) * 128],
                                            x2g[:, k * 128:(k + 1) * 128],
                                            ident[:])
                    x2tg = fp_.tile([128, GRP * 128], F16, tag="x2tg")
                    nc.scalar.activation(x2tg[:, :W * 128], pxg[:, :W * 128],
                                         AF.Identity)
                    for k0 in range(0, W, 2):
                        kw = min(2, W - k0)
                        ph2 = ppx.tile([128, 2 * PAY], F32, tag="tx")
                        for k in range(k0, k0 + kw):
                            nc.tensor.matmul(
                                ph2[:, (k - k0) * PAY:(k - k0 + 1) * PAY],
                                lhsT=x2tg[:, k * 128:(k + 1) * 128],
                                rhs=w2e[:], start=True, stop=True)
                        rowg = fp_.tile([128, 2 * PAY], F16, tag="rowg")
                        nc.scalar.activation(rowg[:, :kw * PAY],
                                             ph2[:, :kw * PAY], AF.Identity)
                        for k in range(k0, k0 + kw):
                            j = live[k]
                            nc.sync.dma_start(
                                out=hx2[j * 128:(j + 1) * 128, :],
                                in_=rowg[:, (k - k0) * PAY:(k - k0 + 1) * PAY])
                else:
                    # t1 + bias -> t1buf (f16); batched exp + per-block sums
                    nc.vector.tensor_tensor(
                        out=t1buf[:, jlo * 128:jlo * 128 + W * 128].rearrange(
                            "p (k e) -> p k e", e=128),
                        in0=t1g[:, :W * 128].rearrange("p (k e) -> p k e", e=128),
                        in1=bass.AP(bt.tensor, bt.offset,
                                    [bt.ap[0], [0, W], [1, 128]]),
                        op=A.add)
                    etg = fp_.tile([128, GRP * 128], F32, tag="etg")
                    nc.scalar.activation(etg[:, :W * 128],
                                         t1buf[:, jlo * 128:jlo * 128 + W * 128],
                                         AF.Exp)
                    nc.vector.tensor_reduce(
                        sumbuf[:, jlo:jlo + W],
                        etg[:, :W * 128].rearrange("p (k e) -> p k e", e=128),
                        axis=mybir.AxisListType.X, op=A.add)
            if layer2:
                # deferred log-softmax tail: one Ln, then bias-apply per block
                lns = cpool.tile([128, NB], F32)
                nc.scalar.activation(lns[:], sumbuf[:], AF.Ln)
                nc.vector.tensor_scalar_mul(lns[:], lns[:], -1.0)
                for j in range(NB):
                    if block_ts[j] == 0:
                        continue
                    to = fp_.tile([128, 128], F32, tag="to")
                    nc.scalar.activation(to[:],
                                         t1buf[:, j * 128:(j + 1) * 128],
                                         AF.Identity, bias=lns[:, j:j + 1])
                    nc.sync.dma_start(out=outp[j * 128:(j + 1) * 128, :],
                                      in_=to[:])
    nc.compile()
    _split_waits(nc, max_waits=1)
    return nc


# ----------------------------------------------------------------------------
# runner

def _rep_heads(a):
    return np.tile(np.asarray(a).reshape(1, -1).astype(np.float32), (128, 1))


def _run(nc, in_maps):
    from concourse.bass_utils import run_bass_kernel_spmd
    return run_bass_kernel_spmd(nc, in_maps, core_ids=list(range(NCORES)),
                                trace=False).results


def _assemble_tab(plan, hx_by_core, core_seg_nodes):
    """hx rows (per-core, perm order) -> full table with sentinels."""
    n = plan["n_nodes"]
    tab = np.zeros((plan["NTAB"], ROW), np.float16)
    tab[plan["chunk_starts"], 128:132] = SENT
    tpos = plan["tpos"]
    for core in range(NCORES):
        nodes = core_seg_nodes[core]
        tab[tpos[nodes], :PAY] = hx_by_core[core]
    return tab


def _tab_from_blocks(plan, hx2_by_core):
    """hx2 rows (per-core, block order) -> full table with sentinels."""
    tab = np.zeros((plan["NTAB"], ROW), np.float16)
    tab[plan["chunk_starts"], 128:132] = SENT
    tpos = plan["tpos"]
    core_nodes = plan["core_nodes"]
    for core in range(NCORES):
        cn = core_nodes[core]
        vm = cn >= 0
        tab[tpos[cn[vm]], :PAY] = np.asarray(hx2_by_core[core])[vm]
    return tab


def _eds_of(plan, tab):
    """per-core [128, NB*4] fp16 ed table in (part, block*4) layout."""
    NB = plan["NB"]
    core_nodes = plan["core_nodes"]
    tpos = plan["tpos"]
    eds = np.zeros((NCORES, 128, NB * 4), np.float16)
    for core in range(NCORES):
        cn = core_nodes[core].reshape(NB, 128)     # [j, p]
        vm = cn >= 0
        vals = np.zeros((NB, 128, 4), np.float16)
        vals[vm] = tab[tpos[cn[vm]], 132:136]
        eds[core] = vals.transpose(1, 0, 2).reshape(128, NB * 4)
    return eds


def run_pipeline(inputs, n_nodes, run=_run):
    edge = np.asarray(inputs["edge"])
    x = np.asarray(inputs["features"], np.float32)
    W1 = np.asarray(inputs["W1"], np.float32)
    a1s = np.asarray(inputs["a1_src"], np.float32)
    a1d = np.asarray(inputs["a1_dst"], np.float32)
    b1 = np.asarray(inputs["b1"], np.float32)
    W2 = np.asarray(inputs["W2"], np.float32)
    a2s = np.asarray(inputs["a2_src"], np.float32)
    a2d = np.asarray(inputs["a2_dst"], np.float32)
    b2 = np.asarray(inputs["b2"], np.float32)

    plan = build_plan(edge, n_nodes)
    NB, NPC = plan["NB"], plan["NPC"]
    core_nodes = plan["core_nodes"]

    # ---- launch 1: hext1 over nodes in table order split across cores
    tord = np.argsort(plan["tpos"])          # nodes in table-row order
    seg = (n_nodes + NCORES - 1) // NCORES
    pad = seg * NCORES - n_nodes
    tord_p = np.concatenate([tord, tord[:pad]]) if pad else tord
    nc1 = build_hext(seg)
    in1, seg_nodes_of = [], []
    for core in range(NCORES):
        seg_nodes = tord_p[core * seg:(core + 1) * seg]
        seg_nodes_of.append(seg_nodes)
        xT = np.ascontiguousarray(x[seg_nodes].astype(np.float16).T)
        in1.append({
            "xT": xT, "Wt": W1.astype(np.float16),
            "as_rep": _rep_heads(a1s), "ad_rep": _rep_heads(a1d),
        })
    res1 = run(nc1, in1)
    hx_by_core = [np.asarray(res1[c]["hx"]) for c in range(NCORES)]
    # dedupe the pad overlap: later writes win, identical rows anyway
    tab1 = _assemble_tab(plan, hx_by_core, seg_nodes_of)
    eds1 = _eds_of(plan, tab1)

    ident = np.eye(128, dtype=np.float16)

    # ---- launch 2: layer-1 message passing + inline h2 table rows
    nc2 = build_msg(plan, layer2=False)
    in2 = []
    for core in range(NCORES):
        in2.append({
            "tab": tab1, "idxs": plan["idx_tab"][core],
            "eds": eds1[core],
            "btile": np.tile(b1.reshape(1, -1), (128, 1)).astype(np.float32),
            "ident": ident, "W2t": W2.astype(np.float16),
            "a2s_rep": _rep_heads(a2s), "a2d_rep": _rep_heads(a2d),
        })
    res2 = run(nc2, in2)
    tab2 = _tab_from_blocks(plan, [res2[c]["hx2"] for c in range(NCORES)])
    eds2 = _eds_of(plan, tab2)

    # ---- launch 3: layer-2 message passing + log_softmax
    nc3 = build_msg(plan, layer2=True)
    in3 = []
    for core in range(NCORES):
        in3.append({
            "tab": tab2, "idxs": plan["idx_tab"][core],
            "eds": eds2[core],
            "btile": np.tile(b2.reshape(1, -1), (128, 1)).astype(np.float32),
            "ident": ident,
        })
    res3 = run(nc3, in3)

    out = np.zeros((n_nodes, H * D_OUT), np.float32)
    for core in range(NCORES):
        cn = core_nodes[core]
        vm = cn >= 0
        out[cn[vm]] = np.asarray(res3[core]["outp"])[vm]
    return out


def kernel(**inputs):
    return run_pipeline(inputs, N_NODES).astype(np.float32)
